# revision 1
# baseline (speedup 1.0000x reference)
"""Trainium2 Bass kernel for DeformableConv2 block (offset/mask conv ->
modulated deformable conv -> SyncBN -> GELU -> residual).

Sharding: data-parallel over batch B=8 across 8 cores (1 image/core),
weights replicated, BN statistics all-reduced (SyncBatchNorm).

Pipeline per core (image b):
  1. offset/mask 3x3 conv as 54 accumulated bf16 matmuls (im2col via
     strided views of a zero-padded image), fp32 PSUM.
  2. Small-tensor math ([27,1024]-shaped) to produce: floor'd sample
     coords, bilinear weights folded with the sigmoid mask (4 weights,
     interleaved in x-pairs), and int16 gather indices in the
     16-partition-wrapped layout ap_gather wants.
  3. GPSIMD ap_gather (d=2) pulls (x[p], x[p+1]) bf16 pairs for the top
     and bottom bilinear rows; DVE combines them with the interleaved
     mask weights (4 tensor ops per tile).
  4. PE contracts w[o,c,k] against the combined samples (bf16, fp32
     PSUM), 512-wide hw blocks.
  5. Per-channel sum/sumsq accumulate via ACT accum_out; [128,12]
     AllReduce across the 8 cores; normalize + erf-GELU + residual.
"""

import sys

sys.path.insert(0, "/opt/trn_rl_repo")

from contextlib import ExitStack

import ml_dtypes
import numpy as np

import concourse.bacc as bacc
import concourse.bass as bass
import concourse.tile as tile
from concourse import mybir
from concourse.bass_utils import run_bass_kernel_spmd

F32 = mybir.dt.float32
BF16 = mybir.dt.bfloat16
I16 = mybir.dt.int16
I32 = mybir.dt.int32
AF = mybir.ActivationFunctionType
OP = mybir.AluOpType

B, C, H, W = 8, 768, 32, 32
CC = C // 128            # 6 channel chunks
HW = H * W               # 1024
K = 9                    # 3x3 taps
PAD = 9                  # sample coords in [-9, 41] -> padded [0, 50]
PADR, PADC = 51, 52
NP = PADR * PADC         # 2652 padded pixels
BLK = 512                # hw block (matmul moving dim)
NB = HW // BLK           # 2
NIDX = K * BLK           # 4608 samples per (block, pair-row)
# k-groups so gather/combine tiles stay small enough for SBUF
KGS = [(0, 3), (3, 6), (6, 9)]
EPS = 1e-5
N_CORES = 8

_CACHE = {}


def _build_program(mock_cc=False):
    nc = bacc.Bacc("TRN2", target_bir_lowering=False)

    # ---- DRAM I/O ----
    xx_d = nc.dram_tensor("xx", [CC, 128, NP * 2], BF16, kind="ExternalInput")
    xres_d = nc.dram_tensor("xres", [CC, 128, HW], F32, kind="ExternalInput")
    wproj_d = nc.dram_tensor("wproj", [K, CC, 128, C], BF16, kind="ExternalInput")
    wom_d = nc.dram_tensor("wom", [K, CC, 128, 27], BF16, kind="ExternalInput")
    bom_d = nc.dram_tensor("bom", [27, 1], F32, kind="ExternalInput")
    gb_d = nc.dram_tensor("gb16", [18, HW], F32, kind="ExternalInput")
    pb_d = nc.dram_tensor("pbias", [CC, 128], F32, kind="ExternalInput")
    gam_d = nc.dram_tensor("gamma2", [CC, 128], F32, kind="ExternalInput")
    bet_d = nc.dram_tensor("beta2", [CC, 128], F32, kind="ExternalInput")
    out_d = nc.dram_tensor("out", [CC, 128, HW], F32, kind="ExternalOutput")

    with tile.TileContext(nc) as tc, ExitStack() as ctx:
        cst = ctx.enter_context(tc.tile_pool(name="cst", bufs=1))
        sm = ctx.enter_context(tc.tile_pool(name="sm", bufs=9))
        pconv = ctx.enter_context(tc.tile_pool(name="pconv", bufs=1, space="PSUM"))
        pmain = ctx.enter_context(tc.tile_pool(name="pmain", bufs=1, space="PSUM"))
        dram = ctx.enter_context(tc.tile_pool(name="dram", bufs=1, space="DRAM"))
        mctx = ExitStack()
        vpool = mctx.enter_context(tc.tile_pool(name="vp", bufs=3))
        rpool = mctx.enter_context(tc.tile_pool(name="rp", bufs=2))
        mpool = mctx.enter_context(tc.tile_pool(name="mp", bufs=2))
        wpool = mctx.enter_context(tc.tile_pool(name="wp", bufs=2))

        # ---- load constants / image ----
        xx = cst.tile([128, CC, NP * 2], BF16)
        for cc_ld in range(CC):
            nc.sync.dma_start(
                out=xx[:, cc_ld], in_=xx_d.ap().transpose([1, 0, 2])[:, cc_ld]
            )
        womsb = cst.tile([128, K, CC, 27], BF16)
        nc.sync.dma_start(out=womsb[:], in_=wom_d.ap().transpose([2, 0, 1, 3]))
        bom = cst.tile([27, 1], F32)
        nc.sync.dma_start(out=bom[:], in_=bom_d[:])
        gb = sm.tile([18, HW], F32, tag="s4")
        nc.sync.dma_start(out=gb[:], in_=gb_d[:])
        pb = cst.tile([128, CC], F32)
        nc.sync.dma_start(out=pb[:], in_=pb_d.ap().transpose([1, 0]))
        gam = cst.tile([128, CC], F32)
        nc.sync.dma_start(out=gam[:], in_=gam_d.ap().transpose([1, 0]))
        bet = cst.tile([128, CC], F32)
        nc.sync.dma_start(out=bet[:], in_=bet_d.ap().transpose([1, 0]))

        # ---- offset/mask conv: psum27[oc, hw] over 54 (cc,k) matmuls ----
        psum27 = pconv.tile([27, HW], F32)
        # padded image view (stride-2 over the interleaved pair tensor)
        xgrid = xx[:].rearrange("p c (n two) -> p c n two", two=2)
        for cc in range(CC):
            for k in range(K):
                ki, kj = k // 3, k % 3
                base = (8 + ki) * PADC + (8 + kj)
                rhs = (
                    xgrid[:, cc, :, 0]
                    .rearrange("p (r c) -> p r c", r=PADR, c=PADC)[
                        :, 8 + ki : 8 + ki + 32, 8 + kj : 8 + kj + 32
                    ]
                )
                del base
                for h in range(2):
                    nc.tensor.matmul(
                        psum27[:, h * BLK : (h + 1) * BLK],
                        lhsT=womsb[:, k, cc, :],
                        rhs=rhs[:, h * 16 : (h + 1) * 16, :],
                        start=(cc == 0 and k == 0),
                        stop=(cc == CC - 1 and k == K - 1),
                    )

        # ---- small-tensor math ----
        # row layout: dy taps at partitions 0-8, dx at 32-40, mask at 64-72
        # (engine APs must start at 32-aligned partitions; DMAs extract the
        # non-zero-based row groups into base-0 tiles)
        omx = sm.tile([27, HW], F32, tag="s4")
        nc.scalar.activation(omx[:], psum27[:], AF.Identity, bias=bom[:])
        doff = sm.tile([18, HW], F32, tag="s4")
        nc.vector.tensor_scalar(doff[:], omx[0:18, :], 8.0, -8.0, OP.min, OP.max)
        s16 = sm.tile([18, HW], F32, tag="s4")
        nc.vector.tensor_tensor(s16[:], doff[:], gb[:], OP.add)
        i32 = sm.tile([18, HW], I32, tag="s4")
        nc.vector.tensor_copy(i32[:], s16[:])
        fint = sm.tile([18, HW], F32, tag="s4")
        nc.vector.tensor_copy(fint[:], i32[:])
        corr = sm.tile([18, HW], F32, tag="s4")
        nc.vector.tensor_tensor(corr[:], fint[:], s16[:], OP.is_gt)
        ffc = sm.tile([18, HW], F32, tag="s4")
        nc.vector.tensor_tensor(ffc[:], fint[:], corr[:], OP.subtract)
        frac = sm.tile([18, HW], F32, tag="s4")
        nc.vector.tensor_tensor(frac[:], s16[:], ffc[:], OP.subtract)
        u1 = sm.tile([18, HW], F32, tag="s4")
        nc.vector.tensor_scalar(u1[:], frac[:], -1.0, 1.0, OP.mult, OP.add)
        # extract x-role and mask rows to partition-base-0 tiles (via DMA)
        frx = sm.tile([9, HW], F32, tag="s4")
        nc.scalar.dma_start(out=frx[:], in_=frac[9:18, :])
        u1x = sm.tile([9, HW], F32, tag="s4")
        nc.scalar.dma_start(out=u1x[:], in_=u1[9:18, :])
        ffx = sm.tile([9, HW], F32, tag="s4")
        nc.scalar.dma_start(out=ffx[:], in_=ffc[9:18, :])
        omm = sm.tile([9, HW], F32, tag="s4")
        nc.scalar.dma_start(out=omm[:], in_=omx[18:27, :])
        m2 = sm.tile([9, HW], F32, tag="s4")
        nc.scalar.activation(m2[:], omm[:], AF.Sigmoid)
        wA = sm.tile([9, HW], F32, tag="s4")
        nc.vector.scalar_tensor_tensor(wA[:], m2[:], 2.0, u1[0:9, :], OP.mult, OP.mult)
        wB = sm.tile([9, HW], F32, tag="s4")
        nc.vector.scalar_tensor_tensor(wB[:], m2[:], 2.0, frac[0:9, :], OP.mult, OP.mult)

        mbT = sm.tile([9, 2 * HW], BF16, tag="s4")
        mbB = sm.tile([9, 2 * HW], BF16, tag="s4")
        mbT2 = mbT[:].rearrange("p (n two) -> p n two", two=2)
        mbB2 = mbB[:].rearrange("p (n two) -> p n two", two=2)
        nc.vector.tensor_tensor(mbT2[:, :, 0], wA[:], u1x[:], OP.mult)
        nc.vector.tensor_tensor(mbT2[:, :, 1], wA[:], frx[:], OP.mult)
        nc.vector.tensor_tensor(mbB2[:, :, 0], wB[:], u1x[:], OP.mult)
        nc.vector.tensor_tensor(mbB2[:, :, 1], wB[:], frx[:], OP.mult)
        mbdram = dram.tile([2, 9, 2 * HW], BF16)
        nc.scalar.dma_start(out=mbdram[0], in_=mbT[:])
        nc.scalar.dma_start(out=mbdram[1], in_=mbB[:])

        # gather indices: p = yf*52 + xf - 371 (pair start in padded image)
        idxf = sm.tile([9, HW], F32, tag="s4")
        nc.vector.scalar_tensor_tensor(
            idxf[:], ffc[0:9, :], 52.0, ffx[:], OP.mult, OP.add
        )
        idxf2 = sm.tile([9, HW], F32, tag="s4")
        nc.vector.tensor_scalar(idxf2[:], idxf[:], -371.0, None, OP.add)
        idx16 = sm.tile([9, HW], I16, tag="s4")
        nc.vector.tensor_copy(idx16[:], idxf2[:])

        # wrapped layout: idxwT[p, s] = flat[16*s + p%16],
        # flat order f = b*4608 + k*512 + hw'
        idxwT = cst.tile([128, K * HW // 16], I16)  # [128, 576]
        # three-hop build of the 16-partition-wrapped index layout:
        # (a) reshape tap row -> [32(h), b, 16(r)]; (b) replicate columns x8;
        # (c) DMA-transpose [32,128] -> [128,32]: wrap + group replication.
        for bb in range(NB):
            for k in range(K):
                eng1 = nc.sync if k % 2 == 0 else nc.scalar
                eng2 = nc.scalar if k % 2 == 0 else nc.sync
                t1w = sm.tile([32, 16], I16, tag="t1w", name="t1w", bufs=2)
                eng1.dma_start(
                    out=t1w[:],
                    in_=idx16[k : k + 1, bb * BLK : (bb + 1) * BLK].rearrange(
                        "o (h r) -> o h r", h=32, r=16
                    ),
                )
                t2w = sm.tile([32, 128], I16, tag="t2w", name="t2w", bufs=4)
                eng2.dma_start(
                    out=t2w[:].rearrange("h (g r) -> h g r", g=8, r=16),
                    in_=t1w[:].unsqueeze(1).broadcast_to((32, 8, 16)),
                )
                nc.sync.dma_start(
                    out=idxwT[:, bb * 288 + k * 32 : bb * 288 + (k + 1) * 32],
                    in_=t2w[:],
                    transpose=True,
                )
        # rebase block-1 indices onto its 36-row source window (rows 15..51)
        idxwB = cst.tile([128, K * HW // 16], I16)
        nc.vector.tensor_scalar(idxwB[:, 0:288], idxwT[:, 0:288], 52, None, OP.add)
        nc.vector.tensor_scalar(
            idxwT[:, 288:576], idxwT[:, 288:576], -780, None, OP.add
        )
        nc.vector.tensor_scalar(
            idxwB[:, 288:576], idxwT[:, 288:576], 52, None, OP.add
        )

        # ---- main loop: gather / combine / matmul ----
        ysb = cst.tile([128, CC, HW], F32)
        stats = cst.tile([128, 4 * CC], F32)  # [S_b0|S_b1|Q_b0|Q_b1]
        sqscr = vpool.tile([128, BLK], F32, tag="vT", name="sqscr")

        for b in range(NB):
            psums = [
                pmain.tile([128, BLK], F32, tag=f"ps{o}", name=f"psum_b{b}_o{o}")
                for o in range(CC)
            ]
            for kg0, kg1 in KGS:
                nk = kg1 - kg0
                ni = nk * BLK
                mrepT = mpool.tile([128, nk, BLK, 2], BF16, tag="mT", name="mrepT")
                nc.scalar.dma_start(
                    out=mrepT[:],
                    in_=mbdram[0][:, b * 2 * BLK : (b + 1) * 2 * BLK]
                    .rearrange("k (h two) -> k h two", two=2)[kg0:kg1]
                    .unsqueeze(0)
                    .broadcast_to((128, nk, BLK, 2)),
                )
                mrepB = mpool.tile([128, nk, BLK, 2], BF16, tag="mB", name="mrepB")
                nc.scalar.dma_start(
                    out=mrepB[:],
                    in_=mbdram[1][:, b * 2 * BLK : (b + 1) * 2 * BLK]
                    .rearrange("k (h two) -> k h two", two=2)[kg0:kg1]
                    .unsqueeze(0)
                    .broadcast_to((128, nk, BLK, 2)),
                )
                for cc in range(CC):
                    rs = 0 if b == 0 else 15 * PADC * 2
                    ne = 36 * PADC
                    vT = vpool.tile([128, 2 * ni], BF16, tag="vT", name="vT")
                    nc.gpsimd.ap_gather(
                        vT[:],
                        xx[:, cc, rs : rs + 2 * ne],
                        idxwT[:, b * (K * 32) + kg0 * 32 : b * (K * 32) + kg1 * 32],
                        channels=128,
                        num_elems=ne,
                        d=2,
                        num_idxs=ni,
                    )
                    vB = vpool.tile([128, 2 * ni], BF16, tag="vB", name="vB")
                    nc.gpsimd.ap_gather(
                        vB[:],
                        xx[:, cc, rs : rs + 2 * ne],
                        idxwB[:, b * (K * 32) + kg0 * 32 : b * (K * 32) + kg1 * 32],
                        channels=128,
                        num_elems=ne,
                        d=2,
                        num_idxs=ni,
                    )
                    # in-place: vT *= mbT ; vB *= mbB ; vT += vB ; R = pairsum
                    vT3 = vT[:].rearrange("p (n two) -> p n two", two=2)
                    vB3 = vB[:].rearrange("p (n two) -> p n two", two=2)
                    nc.vector.tensor_tensor(vT[:], vT[:], mrepT[:].opt(), OP.mult)
                    nc.vector.tensor_tensor(vB[:], vB[:], mrepB[:].opt(), OP.mult)
                    nc.vector.tensor_tensor(vT[:], vT[:], vB[:], OP.add)
                    R = rpool.tile([128, ni], BF16, tag="R", name="R")
                    nc.vector.tensor_tensor(R[:], vT3[:, :, 0], vT3[:, :, 1], OP.add)
                    wt = wpool.tile([128, nk, C], BF16, tag="wt", name="wt")
                    nc.sync.dma_start(
                        out=wt[:], in_=wproj_d[kg0:kg1, cc].transpose([1, 0, 2])
                    )
                    for k in range(kg0, kg1):
                        for o in range(CC):
                            nc.tensor.matmul(
                                psums[o][:],
                                lhsT=wt[:, k - kg0, o * 128 : (o + 1) * 128],
                                rhs=R[:, (k - kg0) * BLK : (k - kg0 + 1) * BLK],
                                start=(cc == 0 and k == 0),
                                stop=(cc == CC - 1 and k == K - 1),
                            )
            for o in range(CC):
                nc.scalar.activation(
                    ysb[:, o, b * BLK : (b + 1) * BLK],
                    psums[o][:],
                    AF.Identity,
                    bias=pb[:, o : o + 1],
                    accum_out=stats[:, b * CC + o : b * CC + o + 1],
                )
                nc.scalar.activation(
                    sqscr[:],
                    ysb[:, o, b * BLK : (b + 1) * BLK],
                    AF.Square,
                    accum_out=stats[:, (2 + b) * CC + o : (2 + b) * CC + o + 1],
                )

        mctx.close()
        opool = ctx.enter_context(tc.tile_pool(name="op", bufs=2))

        # ---- SyncBN stats all-reduce ----
        ssum = sm.tile([128, 2 * CC], F32)
        nc.vector.tensor_tensor(
            ssum[:, 0:CC], stats[:, 0:CC], stats[:, CC : 2 * CC], OP.add
        )
        nc.vector.tensor_tensor(
            ssum[:, CC : 2 * CC],
            stats[:, 2 * CC : 3 * CC],
            stats[:, 3 * CC : 4 * CC],
            OP.add,
        )
        statloc = dram.tile([128, 2 * CC], F32)
        statglob = dram.tile([128, 2 * CC], F32, addr_space="Shared")
        nc.sync.dma_start(out=statloc[:], in_=ssum[:])
        if mock_cc:
            nc.sync.dma_start(out=statglob[:], in_=statloc[:])
        else:
            nc.gpsimd.collective_compute(
                "AllReduce",
                OP.add,
                replica_groups=[list(range(N_CORES))],
                ins=[statloc[:]],
                outs=[statglob[:]],
            )
        gst = sm.tile([128, 2 * CC], F32)
        nc.sync.dma_start(out=gst[:], in_=statglob[:])

        inv_n = 1.0 / (B * HW)
        mean = sm.tile([128, CC], F32)
        nc.vector.tensor_scalar(mean[:], gst[:, 0:CC], inv_n, None, OP.mult)
        ex2 = sm.tile([128, CC], F32)
        nc.vector.tensor_scalar(ex2[:], gst[:, CC : 2 * CC], inv_n, None, OP.mult)
        var = sm.tile([128, CC], F32)
        nc.vector.scalar_tensor_tensor(var[:], mean[:], 1.0, mean[:], OP.mult, OP.mult)
        nc.vector.tensor_tensor(var[:], ex2[:], var[:], OP.subtract)
        epst = sm.tile([128, 1], F32)
        nc.vector.memset(epst[:], EPS)
        std = sm.tile([128, CC], F32)
        nc.scalar.activation(std[:], var[:], AF.Sqrt, bias=epst[:])
        inv = sm.tile([128, CC], F32)
        nc.vector.reciprocal(inv[:], std[:])
        scl = sm.tile([128, CC], F32)
        nc.vector.tensor_tensor(scl[:], gam[:], inv[:], OP.mult)
        sft = sm.tile([128, CC], F32)
        nc.vector.tensor_tensor(sft[:], mean[:], scl[:], OP.mult)
        nc.vector.tensor_tensor(sft[:], bet[:], sft[:], OP.subtract)

        # ---- normalize + erf-GELU + residual ----
        for cc in range(CC):
            for hb in range(NB):
                hs = slice(hb * BLK, (hb + 1) * BLK)
                xres = opool.tile([128, BLK], F32, tag="xr", name="xres")
                nc.scalar.dma_start(out=xres[:], in_=xres_d[cc][:, hs])
                yn = opool.tile([128, BLK], F32, tag="yn", name="yn")
                nc.vector.tensor_scalar(
                    yn[:],
                    ysb[:, cc, hs],
                    scl[:, cc : cc + 1],
                    sft[:, cc : cc + 1],
                    OP.mult,
                    OP.add,
                )
                erf = opool.tile([128, BLK], F32, tag="erf", name="erf")
                nc.scalar.activation(
                    erf[:], yn[:], AF.Erf, scale=float(1.0 / np.sqrt(2.0))
                )
                nc.vector.tensor_scalar(erf[:], erf[:], 0.5, 0.5, OP.mult, OP.add)
                nc.vector.tensor_tensor(erf[:], yn[:], erf[:], OP.mult)
                nc.vector.tensor_tensor(erf[:], erf[:], xres[:], OP.add)
                nc.scalar.dma_start(out=out_d[cc][:, hs], in_=erf[:])

    nc.compile()
    return nc


def _host_prep(inputs):
    x = np.asarray(inputs["x"], np.float32)
    proj_w = np.asarray(inputs["proj_w"], np.float32)
    proj_b = np.asarray(inputs["proj_b"], np.float32)
    offset_w = np.asarray(inputs["offset_w"], np.float32)
    offset_b = np.asarray(inputs["offset_b"], np.float32)
    mask_w = np.asarray(inputs["mask_w"], np.float32)
    mask_b = np.asarray(inputs["mask_b"], np.float32)
    gamma = np.asarray(inputs["gamma"], np.float32)
    beta = np.asarray(inputs["beta"], np.float32)

    bf = ml_dtypes.bfloat16
    # zero-padded image + interleaved (x[p], x[p+1]) pairs
    xpad = np.zeros((B, C, PADR, PADC), np.float32)
    xpad[:, :, PAD : PAD + H, PAD : PAD + W] = x
    xf = xpad.reshape(B, C, NP)
    xx = np.zeros((B, C, NP, 2), np.float32)
    xx[:, :, :, 0] = xf
    xx[:, :, :-1, 1] = xf[:, :, 1:]
    xx_bf = xx.astype(bf).reshape(B, CC, 128, NP * 2)

    xres = x.reshape(B, CC, 128, HW).astype(np.float32)

    # proj weights -> [k, cc, c128, o]
    wproj = (
        proj_w.reshape(C, C, K)
        .transpose(2, 1, 0)
        .reshape(K, CC, 128, C)
        .astype(bf)
    )
    # dy taps rows 0-8, dx rows 9-17, mask rows 18-26
    ow = offset_w.reshape(K, 2, C, K)
    om_w = np.concatenate([ow[:, 0], ow[:, 1], mask_w.reshape(K, C, K)], axis=0)
    wom = om_w.transpose(2, 1, 0).reshape(K, CC, 128, 27).astype(bf)
    ob = offset_b.reshape(K, 2)
    bom = np.concatenate([ob[:, 0], ob[:, 1], mask_b]).reshape(27, 1).astype(np.float32)

    hh, ww = np.meshgrid(np.arange(H), np.arange(W), indexing="ij")
    gb = np.zeros((18, HW), np.float32)
    for k in range(K):
        ki, kj = k // 3, k % 3
        gb[k] = (hh + ki - 1 + 16).reshape(-1)
        gb[9 + k] = (ww + kj - 1 + 16).reshape(-1)

    pb = proj_b.reshape(CC, 128).astype(np.float32)
    gam2 = gamma.reshape(CC, 128).astype(np.float32)
    bet2 = beta.reshape(CC, 128).astype(np.float32)

    shared = {
        "wproj": wproj,
        "wom": wom,
        "bom": bom,
        "gb16": gb,
        "pbias": pb,
        "gamma2": gam2,
        "beta2": bet2,
    }
    in_maps = []
    for b in range(B):
        m = dict(shared)
        m["xx"] = xx_bf[b]
        m["xres"] = xres[b]
        in_maps.append(m)
    return in_maps


def kernel(**inputs):
    if "nc" not in _CACHE:
        _CACHE["nc"] = _build_program()
    nc = _CACHE["nc"]
    in_maps = _host_prep(inputs)
    res = run_bass_kernel_spmd(nc, in_maps, list(range(N_CORES)))
    out = np.stack([r["out"].reshape(C, H, W) for r in res.results])
    return out.astype(np.float32)


if __name__ == "__main__":
    nc = _build_program()
    print("program built OK;", len(nc.m.functions[0].blocks), "blocks")



# revision 7
# speedup vs baseline: 4.8749x; 4.8749x over previous
"""Trainium2 Bass kernel for DeformableConv2 block (offset/mask conv ->
modulated deformable conv -> SyncBN -> GELU -> residual).

Sharding: data-parallel over batch B=8 across 8 cores (1 image/core),
weights replicated, BN statistics all-reduced (SyncBatchNorm).

Pipeline per core (image b):
  1. x loaded once as [128, 6, 34, 34] zero-padded bf16 (xp). PE
     transposes build a pixel-major padded DRAM copy xd[2688, 768]
     (rows = padded pixels, 768 channels contiguous per row).
  2. offset/mask 3x3 conv as 54 accumulated bf16 matmuls from strided
     views of xp, fp32 PSUM [27, 1024].
  3. Small-tensor math produces: int16 gather indices (top-left padded
     pixel of each bilinear 2x2 patch) in the 16-partition-wrapped
     layout, and 4 mask-folded bilinear weights (a00,a01,a10,a11) in
     DRAM for per-block broadcast.
  4. HWDGE dma_gather(transpose=True, elem=1536, step=768): one call
     per (half-image, tap, top/bottom) pulls 512 horizontal pixel
     pairs across all 768 channels, transposed to channel-partition
     layout [128, 12, 512]. This runs on the DMA engines at HBM
     bandwidth instead of the Q7 cores (the old ap_gather bottleneck).
  5. DVE combine (5 big ops per (half, tap)) folds the 4 bilinear
     weights + mask: R[128, 6, 512] bf16.
  6. PE contracts wproj[o,c,k] against R: 36 matmuls per (half, tap),
     fp32 PSUM, 6 banks (one per 128-wide output-channel chunk).
  7. Per-channel sum/sumsq via ACT accum_out; [128,12] AllReduce
     across the 8 cores; normalize + erf-GELU + residual (from xp).
"""

import sys

sys.path.insert(0, "/opt/trn_rl_repo")

from contextlib import ExitStack

import ml_dtypes
import numpy as np

import bass_rust
import concourse.bacc as bacc
import concourse.bass as bass
import concourse.tile as tile
from concourse import mybir
from concourse.bass_utils import run_bass_kernel_spmd

F32 = mybir.dt.float32
BF16 = mybir.dt.bfloat16
I16 = mybir.dt.int16
I32 = mybir.dt.int32
AF = mybir.ActivationFunctionType
OP = mybir.AluOpType

B, C, H, W = 8, 768, 32, 32
CC = C // 128            # 6 channel chunks
HW = H * W               # 1024
K = 9                    # 3x3 taps
PAD = 9                  # sample coords in [-9, 40] -> padded rows [0, 50]
PADC = 52                # padded row stride (51 cols + 1 for x+1 pair)
XD_ROWS = 2688           # 21*128 (>= 51*52 + slack for +52 B-view)
VIEW_ROWS = 2600         # gather view row count (> max idx 2597)
BLK = 512                # hw block (matmul moving dim / gather call size)
NB = HW // BLK           # 2
KGS = [(0, 3), (3, 6), (6, 9)]
EPS = 1e-5
N_CORES = 8

_CACHE = {}


def _build_program(mock_cc=False):
    nc = bacc.Bacc("TRN2", target_bir_lowering=False)

    # ---- DRAM I/O ----
    x_d = nc.dram_tensor("x16", [CC, 128, HW], BF16, kind="ExternalInput")
    wproj_d = nc.dram_tensor("wproj", [K, CC, 128, C], BF16, kind="ExternalInput")
    wom_d = nc.dram_tensor("wom", [K, CC, 128, 27], BF16, kind="ExternalInput")
    bom_d = nc.dram_tensor("bom", [27, 1], F32, kind="ExternalInput")
    gb_d = nc.dram_tensor("gb16", [18, HW], F32, kind="ExternalInput")
    pb_d = nc.dram_tensor("pbias", [CC, 128], F32, kind="ExternalInput")
    gam_d = nc.dram_tensor("gamma2", [CC, 128], F32, kind="ExternalInput")
    bet_d = nc.dram_tensor("beta2", [CC, 128], F32, kind="ExternalInput")
    id_d = nc.dram_tensor("ident", [128, 128], BF16, kind="ExternalInput")
    out_d = nc.dram_tensor("out", [CC, 128, HW], BF16, kind="ExternalOutput")

    with tile.TileContext(nc) as tc, ExitStack() as ctx:
        cst = ctx.enter_context(tc.tile_pool(name="cst", bufs=1))
        dram = ctx.enter_context(tc.tile_pool(name="dram", bufs=1, space="DRAM"))
        actx = ExitStack()  # phase A/B scratch (closed before main loop)
        sm = actx.enter_context(tc.tile_pool(name="sm", bufs=9))
        pconv = actx.enter_context(tc.tile_pool(name="pconv", bufs=1, space="PSUM"))
        pt = actx.enter_context(tc.tile_pool(name="pt", bufs=2, space="PSUM"))
        xtp = actx.enter_context(tc.tile_pool(name="xtp", bufs=2))

        # ---- constants / image ----
        xsb = cst.tile([128, CC, HW], BF16)
        nc.sync.dma_start(out=xsb[:], in_=x_d.ap().transpose([1, 0, 2]))
        xp = cst.tile([128, CC, 34, 34], BF16)
        nc.vector.memset(xp[:], 0.0)
        for cc_ld in range(CC):
            nc.sync.dma_start(
                out=xp[:, cc_ld, 1:33, 1:33],
                in_=x_d.ap().transpose([1, 0, 2])[:, cc_ld].rearrange(
                    "p (y x) -> p y x", y=32
                ),
            )
        womsb = cst.tile([128, K, CC, 27], BF16)
        nc.sync.dma_start(out=womsb[:], in_=wom_d.ap().transpose([2, 0, 1, 3]))
        bom = cst.tile([27, 1], F32)
        nc.sync.dma_start(out=bom[:], in_=bom_d[:])
        gb = cst.tile([18, HW], F32)
        nc.sync.dma_start(out=gb[:], in_=gb_d[:])
        pb = cst.tile([128, CC], F32)
        nc.sync.dma_start(out=pb[:], in_=pb_d.ap().transpose([1, 0]))
        gam = cst.tile([128, CC], F32)
        nc.sync.dma_start(out=gam[:], in_=gam_d.ap().transpose([1, 0]))
        bet = cst.tile([128, CC], F32)
        nc.sync.dma_start(out=bet[:], in_=bet_d.ap().transpose([1, 0]))
        ident = cst.tile([128, 128], BF16)
        nc.sync.dma_start(out=ident[:], in_=id_d[:])

        # ---- offset/mask conv: psum27[oc, hw] over 54 (cc,k) matmuls ----
        psum27 = pconv.tile([27, HW], F32)
        for cc in range(CC):
            for k in range(K):
                ki, kj = k // 3, k % 3
                for h in range(2):
                    nc.tensor.matmul(
                        psum27[:, h * BLK : (h + 1) * BLK],
                        lhsT=womsb[:, k, cc, :],
                        rhs=xp[:, cc, ki + h * 16 : ki + h * 16 + 16, kj : kj + 32],
                        start=(cc == 0 and k == 0),
                        stop=(cc == CC - 1 and k == K - 1),
                    )

        # ---- pixel-major padded DRAM image xd[2688, 768] ----
        xd = dram.tile([XD_ROWS, C], BF16)
        zt = cst.tile([128, C], BF16)
        nc.vector.memset(zt[:], 0.0)
        for j in range(XD_ROWS // 128):
            nc.scalar.dma_start(out=xd[j * 128 : (j + 1) * 128, :], in_=zt[:])
        for pb8 in range(8):
            xt = xtp.tile([128, C], BF16, tag="xt", name=f"xt{pb8}")
            for cc in range(CC):
                pst = pt.tile([128, 128], BF16, tag="pst", name=f"pst{pb8}_{cc}")
                nc.tensor.transpose(
                    pst[:],
                    xsb[:, cc, pb8 * 128 : (pb8 + 1) * 128],
                    ident[:],
                )
                nc.scalar.activation(
                    xt[:, cc * 128 : (cc + 1) * 128], pst[:], AF.Identity
                )
            for a in range(4):
                r0 = (PAD + pb8 * 4 + a) * PADC + PAD
                nc.sync.dma_start(
                    out=xd[r0 : r0 + 32, :], in_=xt[a * 32 : (a + 1) * 32, :]
                )

        # ---- small-tensor math ----
        # psum rows: dy taps 0-8, dx taps 9-17, mask 18-26
        omx = sm.tile([27, HW], F32, tag="s4")
        nc.scalar.activation(omx[:], psum27[:], AF.Identity, bias=bom[:])
        doff = sm.tile([18, HW], F32, tag="s4")
        nc.vector.tensor_scalar(doff[:], omx[0:18, :], 8.0, -8.0, OP.min, OP.max)
        s16 = sm.tile([18, HW], F32, tag="s4")
        nc.vector.tensor_tensor(s16[:], doff[:], gb[:], OP.add)
        i32 = sm.tile([18, HW], I32, tag="s4")
        nc.vector.tensor_copy(i32[:], s16[:])
        fint = sm.tile([18, HW], F32, tag="s4")
        nc.vector.tensor_copy(fint[:], i32[:])
        corr = sm.tile([18, HW], F32, tag="s4")
        nc.vector.tensor_tensor(corr[:], fint[:], s16[:], OP.is_gt)
        ffc = sm.tile([18, HW], F32, tag="s4")
        nc.vector.tensor_tensor(ffc[:], fint[:], corr[:], OP.subtract)
        frac = sm.tile([18, HW], F32, tag="s4")
        nc.vector.tensor_tensor(frac[:], s16[:], ffc[:], OP.subtract)
        u1 = sm.tile([18, HW], F32, tag="s4")
        nc.vector.tensor_scalar(u1[:], frac[:], -1.0, 1.0, OP.mult, OP.add)
        # extract x-role rows to partition-base-0 tiles (engine APs must
        # start at 32-aligned partitions; DMA re-bases them)
        frx = sm.tile([9, HW], F32, tag="s4")
        nc.scalar.dma_start(out=frx[:], in_=frac[9:18, :])
        u1x = sm.tile([9, HW], F32, tag="s4")
        nc.scalar.dma_start(out=u1x[:], in_=u1[9:18, :])
        ffx = sm.tile([9, HW], F32, tag="s4")
        nc.scalar.dma_start(out=ffx[:], in_=ffc[9:18, :])
        omm = sm.tile([9, HW], F32, tag="s4")
        nc.scalar.dma_start(out=omm[:], in_=omx[18:27, :])
        m2 = sm.tile([9, HW], F32, tag="s4")
        nc.scalar.activation(m2[:], omm[:], AF.Sigmoid)
        wA = sm.tile([9, HW], F32, tag="s4")
        nc.vector.scalar_tensor_tensor(wA[:], m2[:], 2.0, u1[0:9, :], OP.mult, OP.mult)
        wB = sm.tile([9, HW], F32, tag="s4")
        nc.vector.scalar_tensor_tensor(wB[:], m2[:], 2.0, frac[0:9, :], OP.mult, OP.mult)
        a00 = sm.tile([9, HW], BF16, tag="s4")
        nc.vector.tensor_tensor(a00[:], wA[:], u1x[:], OP.mult)
        a01 = sm.tile([9, HW], BF16, tag="s4")
        nc.vector.tensor_tensor(a01[:], wA[:], frx[:], OP.mult)
        a10 = sm.tile([9, HW], BF16, tag="s4")
        nc.vector.tensor_tensor(a10[:], wB[:], u1x[:], OP.mult)
        a11 = sm.tile([9, HW], BF16, tag="s4")
        nc.vector.tensor_tensor(a11[:], wB[:], frx[:], OP.mult)
        mwdT = dram.tile([2, K, HW], BF16)
        mwdB = dram.tile([2, K, HW], BF16)
        nc.scalar.dma_start(out=mwdT[0], in_=a00[:])
        nc.scalar.dma_start(out=mwdT[1], in_=a01[:])
        nc.scalar.dma_start(out=mwdB[0], in_=a10[:])
        nc.scalar.dma_start(out=mwdB[1], in_=a11[:])

        # gather index: top-left padded pixel = ffc_y*52 + ffc_x
        idxf = sm.tile([9, HW], F32, tag="s4")
        nc.vector.scalar_tensor_tensor(
            idxf[:], ffc[0:9, :], float(PADC), ffx[:], OP.mult, OP.add
        )
        idx16 = sm.tile([9, HW], I16, tag="s4")
        nc.vector.tensor_copy(idx16[:], idxf[:])

        # wrapped layout: idxw[p, s] = flat[16*s + p%16], replicated x8;
        # column blocks of 32 per (b2, k).
        idxw = cst.tile([128, NB * K * 32], I16)
        for b2 in range(NB):
            for k in range(K):
                eng1 = nc.sync if k % 2 == 0 else nc.scalar
                eng2 = nc.scalar if k % 2 == 0 else nc.sync
                t1w = sm.tile([32, 16], I16, tag="t1w", name="t1w", bufs=2)
                eng1.dma_start(
                    out=t1w[:],
                    in_=idx16[k : k + 1, b2 * BLK : (b2 + 1) * BLK].rearrange(
                        "o (h r) -> o h r", h=32, r=16
                    ),
                )
                t2w = sm.tile([32, 128], I16, tag="t2w", name="t2w", bufs=4)
                eng2.dma_start(
                    out=t2w[:].rearrange("h (g r) -> h g r", g=8, r=16),
                    in_=t1w[:].unsqueeze(1).broadcast_to((32, 8, 16)),
                )
                cb = (b2 * K + k) * 32
                nc.sync.dma_start(
                    out=idxw[:, cb : cb + 32], in_=t2w[:], transpose=True
                )

        actx.close()

        # gather source views: rows of 1536 elems (2 pixels) at stride 768
        srcT = xd[:].copy()
        srcT.ap = bass_rust.VecI64Pair([(C, VIEW_ROWS), (1, 2 * C)])
        srcB = xd[PADC:, :].copy()
        srcB.ap = bass_rust.VecI64Pair([(C, VIEW_ROWS), (1, 2 * C)])

        mctx = ExitStack()
        vpool = mctx.enter_context(tc.tile_pool(name="vp", bufs=2))
        rpool = mctx.enter_context(tc.tile_pool(name="rp", bufs=2))
        mpool = mctx.enter_context(tc.tile_pool(name="mp", bufs=2))
        wpool = mctx.enter_context(tc.tile_pool(name="wp", bufs=2))
        pmain = mctx.enter_context(tc.tile_pool(name="pmain", bufs=1, space="PSUM"))

        ysb = cst.tile([128, CC, HW], BF16)
        stats = cst.tile([128, 4 * CC], F32)  # [S_b0|S_b1|Q_b0|Q_b1]
        sqscr = cst.tile([128, BLK], F32)

        # ---- main loop: gather / combine / matmul ----
        for b2 in range(NB):
            psums = [
                pmain.tile([128, BLK], F32, tag=f"ps{o}", name=f"psum_b{b2}_o{o}")
                for o in range(CC)
            ]
            for kg0, kg1 in KGS:
                wt = wpool.tile([128, 3, CC, C], BF16, tag="wt", name="wt")
                nc.sync.dma_start(
                    out=wt[:], in_=wproj_d[kg0:kg1].transpose([2, 0, 1, 3])
                )
                mbT = mpool.tile([128, 3, 2, BLK], BF16, tag="mT", name="mbT")
                mbB = mpool.tile([128, 3, 2, BLK], BF16, tag="mB", name="mbB")
                for j in range(2):
                    nc.scalar.dma_start(
                        out=mbT[:, :, j, :],
                        in_=mwdT[j, kg0:kg1, b2 * BLK : (b2 + 1) * BLK]
                        .unsqueeze(0)
                        .broadcast_to((128, 3, BLK)),
                    )
                    nc.scalar.dma_start(
                        out=mbB[:, :, j, :],
                        in_=mwdB[j, kg0:kg1, b2 * BLK : (b2 + 1) * BLK]
                        .unsqueeze(0)
                        .broadcast_to((128, 3, BLK)),
                    )
                for k in range(kg0, kg1):
                    cb = (b2 * K + k) * 32
                    vT = vpool.tile([128, 2, CC, BLK], BF16, tag="vT", name="vT")
                    nc.gpsimd.dma_gather(
                        vT[:].rearrange("p a c n -> p (a c) n"),
                        srcT, idxw[:, cb : cb + 32],
                        BLK, BLK, 2 * C, elem_step=C, transpose=True,
                    )
                    vB = vpool.tile([128, 2, CC, BLK], BF16, tag="vB", name="vB")
                    nc.gpsimd.dma_gather(
                        vB[:].rearrange("p a c n -> p (a c) n"),
                        srcB, idxw[:, cb : cb + 32],
                        BLK, BLK, 2 * C, elem_step=C, transpose=True,
                    )
                    mTk = (
                        mbT[:, k - kg0]
                        .unsqueeze(2)
                        .broadcast_to((128, 2, CC, BLK))
                    )
                    mBk = (
                        mbB[:, k - kg0]
                        .unsqueeze(2)
                        .broadcast_to((128, 2, CC, BLK))
                    )
                    nc.vector.tensor_tensor(vT[:], vT[:], mTk, OP.mult)
                    nc.vector.tensor_tensor(vB[:], vB[:], mBk, OP.mult)
                    R = rpool.tile([128, CC, BLK], BF16, tag="R", name="R")
                    nc.vector.tensor_tensor(R[:], vT[:, 0], vT[:, 1], OP.add)
                    tmpB = rpool.tile([128, CC, BLK], BF16, tag="tB", name="tB")
                    nc.vector.tensor_tensor(tmpB[:], vB[:, 0], vB[:, 1], OP.add)
                    nc.vector.tensor_tensor(R[:], R[:], tmpB[:], OP.add)
                    for cc in range(CC):
                        for o in range(CC):
                            nc.tensor.matmul(
                                psums[o][:],
                                lhsT=wt[:, k - kg0, cc, o * 128 : (o + 1) * 128],
                                rhs=R[:, cc, :],
                                start=(k == 0 and cc == 0),
                                stop=(k == K - 1 and cc == CC - 1),
                            )
            for o in range(CC):
                nc.scalar.activation(
                    ysb[:, o, b2 * BLK : (b2 + 1) * BLK],
                    psums[o][:],
                    AF.Identity,
                    bias=pb[:, o : o + 1],
                    accum_out=stats[:, b2 * CC + o : b2 * CC + o + 1],
                )
                nc.scalar.activation(
                    sqscr[:],
                    ysb[:, o, b2 * BLK : (b2 + 1) * BLK],
                    AF.Square,
                    accum_out=stats[:, (2 + b2) * CC + o : (2 + b2) * CC + o + 1],
                )

        mctx.close()
        opool = ctx.enter_context(tc.tile_pool(name="op", bufs=2))
        smf = ctx.enter_context(tc.tile_pool(name="smf", bufs=4))

        # ---- SyncBN stats all-reduce ----
        ssum = smf.tile([128, 2 * CC], F32, tag="f")
        nc.vector.tensor_tensor(
            ssum[:, 0:CC], stats[:, 0:CC], stats[:, CC : 2 * CC], OP.add
        )
        nc.vector.tensor_tensor(
            ssum[:, CC : 2 * CC],
            stats[:, 2 * CC : 3 * CC],
            stats[:, 3 * CC : 4 * CC],
            OP.add,
        )
        statloc = dram.tile([128, 2 * CC], F32)
        statglob = dram.tile([128, 2 * CC], F32, addr_space="Shared")
        nc.sync.dma_start(out=statloc[:], in_=ssum[:])
        if mock_cc:
            nc.sync.dma_start(out=statglob[:], in_=statloc[:])
        else:
            nc.gpsimd.collective_compute(
                "AllReduce",
                OP.add,
                replica_groups=[list(range(N_CORES))],
                ins=[statloc[:]],
                outs=[statglob[:]],
            )
        gst = smf.tile([128, 2 * CC], F32, tag="f")
        nc.sync.dma_start(out=gst[:], in_=statglob[:])

        inv_n = 1.0 / (B * HW)
        mean = smf.tile([128, CC], F32, tag="f")
        nc.vector.tensor_scalar(mean[:], gst[:, 0:CC], inv_n, None, OP.mult)
        ex2 = smf.tile([128, CC], F32, tag="f")
        nc.vector.tensor_scalar(ex2[:], gst[:, CC : 2 * CC], inv_n, None, OP.mult)
        var = smf.tile([128, CC], F32, tag="f")
        nc.vector.scalar_tensor_tensor(var[:], mean[:], 1.0, mean[:], OP.mult, OP.mult)
        nc.vector.tensor_tensor(var[:], ex2[:], var[:], OP.subtract)
        epst = smf.tile([128, 1], F32, tag="f")
        nc.vector.memset(epst[:], EPS)
        std = smf.tile([128, CC], F32, tag="f")
        nc.scalar.activation(std[:], var[:], AF.Sqrt, bias=epst[:])
        inv = smf.tile([128, CC], F32, tag="f")
        nc.vector.reciprocal(inv[:], std[:])
        scl = smf.tile([128, CC], F32, tag="f")
        nc.vector.tensor_tensor(scl[:], gam[:], inv[:], OP.mult)
        sft = smf.tile([128, CC], F32, tag="f")
        nc.vector.tensor_tensor(sft[:], mean[:], scl[:], OP.mult)
        nc.vector.tensor_tensor(sft[:], bet[:], sft[:], OP.subtract)

        # ---- normalize + erf-GELU + residual ----
        for cc in range(CC):
            for hb in range(NB):
                hs = slice(hb * BLK, (hb + 1) * BLK)
                yn = opool.tile([128, BLK], F32, tag="yn", name="yn")
                nc.vector.tensor_scalar(
                    yn[:],
                    ysb[:, cc, hs],
                    scl[:, cc : cc + 1],
                    sft[:, cc : cc + 1],
                    OP.mult,
                    OP.add,
                )
                erf = opool.tile([128, BLK], F32, tag="erf", name="erf")
                nc.scalar.activation(
                    erf[:], yn[:], AF.Erf, scale=float(1.0 / np.sqrt(2.0))
                )
                nc.vector.tensor_scalar(erf[:], erf[:], 0.5, 0.5, OP.mult, OP.add)
                nc.vector.tensor_tensor(erf[:], yn[:], erf[:], OP.mult)
                xr32 = opool.tile([128, BLK], F32, tag="xr", name="xr")
                nc.scalar.activation(
                    xr32[:],
                    xp[:, cc, 1 + hb * 16 : 1 + hb * 16 + 16, 1:33],
                    AF.Identity,
                )
                ob = opool.tile([128, BLK], BF16, tag="ob", name="ob")
                nc.vector.tensor_tensor(ob[:], erf[:], xr32[:], OP.add)
                nc.scalar.dma_start(out=out_d[cc][:, hs], in_=ob[:])

    nc.compile()
    return nc


def _host_prep(inputs):
    x = np.asarray(inputs["x"], np.float32)
    proj_w = np.asarray(inputs["proj_w"], np.float32)
    proj_b = np.asarray(inputs["proj_b"], np.float32)
    offset_w = np.asarray(inputs["offset_w"], np.float32)
    offset_b = np.asarray(inputs["offset_b"], np.float32)
    mask_w = np.asarray(inputs["mask_w"], np.float32)
    mask_b = np.asarray(inputs["mask_b"], np.float32)
    gamma = np.asarray(inputs["gamma"], np.float32)
    beta = np.asarray(inputs["beta"], np.float32)

    bf = ml_dtypes.bfloat16
    x16 = x.reshape(B, CC, 128, HW).astype(bf)

    # proj weights -> [k, cc, c128, o]
    wproj = (
        proj_w.reshape(C, C, K)
        .transpose(2, 1, 0)
        .reshape(K, CC, 128, C)
        .astype(bf)
    )
    # dy taps rows 0-8, dx rows 9-17, mask rows 18-26
    ow = offset_w.reshape(K, 2, C, K)
    om_w = np.concatenate([ow[:, 0], ow[:, 1], mask_w.reshape(K, C, K)], axis=0)
    wom = om_w.transpose(2, 1, 0).reshape(K, CC, 128, 27).astype(bf)
    ob = offset_b.reshape(K, 2)
    bom = np.concatenate([ob[:, 0], ob[:, 1], mask_b]).reshape(27, 1).astype(np.float32)

    hh, ww = np.meshgrid(np.arange(H), np.arange(W), indexing="ij")
    gb = np.zeros((18, HW), np.float32)
    for k in range(K):
        ki, kj = k // 3, k % 3
        gb[k] = (hh + ki - 1 + PAD).reshape(-1)
        gb[9 + k] = (ww + kj - 1 + PAD).reshape(-1)

    pb = proj_b.reshape(CC, 128).astype(np.float32)
    gam2 = gamma.reshape(CC, 128).astype(np.float32)
    bet2 = beta.reshape(CC, 128).astype(np.float32)
    ident = np.eye(128, dtype=bf)

    shared = {
        "wproj": wproj,
        "wom": wom,
        "bom": bom,
        "gb16": gb,
        "pbias": pb,
        "gamma2": gam2,
        "beta2": bet2,
        "ident": ident,
    }
    in_maps = []
    for b in range(B):
        m = dict(shared)
        m["x16"] = x16[b]
        in_maps.append(m)
    return in_maps


def kernel(**inputs):
    if "nc" not in _CACHE:
        _CACHE["nc"] = _build_program()
    nc = _CACHE["nc"]
    in_maps = _host_prep(inputs)
    res = run_bass_kernel_spmd(nc, in_maps, list(range(N_CORES)))
    out = np.stack(
        [r["out"].astype(np.float32).reshape(C, H, W) for r in res.results]
    )
    return out


if __name__ == "__main__":
    nc = _build_program()
    print("program built OK;", len(nc.m.functions[0].blocks), "blocks")


# revision 9
# speedup vs baseline: 5.1523x; 1.0569x over previous
"""Trainium2 Bass kernel for DeformableConv2 block (offset/mask conv ->
modulated deformable conv -> SyncBN -> GELU -> residual).

Sharding: data-parallel over batch B=8 across 8 cores (1 image/core),
weights replicated, BN statistics all-reduced (SyncBatchNorm).

Pipeline per core (image b):
  1. x loaded once as [128, 6, 34, 34] zero-padded bf16 (xp). PE
     transposes build a pixel-major padded DRAM copy xd[2688, 768]
     (rows = padded pixels, 768 channels contiguous per row).
  2. offset/mask 3x3 conv as 54 accumulated bf16 matmuls from strided
     views of xp, fp32 PSUM [27, 1024].
  3. Small-tensor math produces: int16 gather indices (top-left padded
     pixel of each bilinear 2x2 patch) in the 16-partition-wrapped
     layout, and 4 mask-folded bilinear weights (a00,a01,a10,a11) in
     DRAM for per-block broadcast.
  4. HWDGE dma_gather(transpose=True, elem=1536, step=768): one call
     per (half-image, tap, top/bottom) pulls 512 horizontal pixel
     pairs across all 768 channels, transposed to channel-partition
     layout [128, 12, 512]. This runs on the DMA engines at HBM
     bandwidth instead of the Q7 cores (the old ap_gather bottleneck).
  5. DVE combine (5 big ops per (half, tap)) folds the 4 bilinear
     weights + mask: R[128, 6, 512] bf16.
  6. PE contracts wproj[o,c,k] against R: 36 matmuls per (half, tap),
     fp32 PSUM, 6 banks (one per 128-wide output-channel chunk).
  7. Per-channel sum/sumsq via ACT accum_out; [128,12] AllReduce
     across the 8 cores; normalize + erf-GELU + residual (from xp).
"""

import sys

sys.path.insert(0, "/opt/trn_rl_repo")

from contextlib import ExitStack

import ml_dtypes
import numpy as np

import bass_rust
import concourse.bacc as bacc
import concourse.bass as bass
import concourse.tile as tile
from concourse import mybir
from concourse.bass_utils import run_bass_kernel_spmd

F32 = mybir.dt.float32
BF16 = mybir.dt.bfloat16
I16 = mybir.dt.int16
I32 = mybir.dt.int32
AF = mybir.ActivationFunctionType
OP = mybir.AluOpType

B, C, H, W = 8, 768, 32, 32
CC = C // 128            # 6 channel chunks
HW = H * W               # 1024
K = 9                    # 3x3 taps
PAD = 9                  # sample coords in [-9, 40] -> padded rows [0, 50]
PADC = 52                # padded row stride (51 cols + 1 for x+1 pair)
XD_ROWS = 2688           # 21*128 (>= 51*52 + slack for +52 B-view)
VIEW_ROWS = 2600         # gather view row count (> max idx 2597)
BLK = 512                # hw block (matmul moving dim / gather call size)
NB = HW // BLK           # 2
KGS = [(0, 3), (3, 6), (6, 9)]
EPS = 1e-5
N_CORES = 8

_CACHE = {}


def _build_program(mock_cc=False):
    nc = bacc.Bacc("TRN2", target_bir_lowering=False)

    # ---- DRAM I/O ----
    x_d = nc.dram_tensor("x16", [CC, 128, HW], BF16, kind="ExternalInput")
    wproj_d = nc.dram_tensor("wproj", [K, CC, 128, C], BF16, kind="ExternalInput")
    wom_d = nc.dram_tensor("wom", [K, CC, 128, 27], BF16, kind="ExternalInput")
    bom_d = nc.dram_tensor("bom", [27, 1], F32, kind="ExternalInput")
    gb_d = nc.dram_tensor("gb16", [18, HW], F32, kind="ExternalInput")
    pb_d = nc.dram_tensor("pbias", [CC, 128], F32, kind="ExternalInput")
    gam_d = nc.dram_tensor("gamma2", [CC, 128], F32, kind="ExternalInput")
    bet_d = nc.dram_tensor("beta2", [CC, 128], F32, kind="ExternalInput")
    id_d = nc.dram_tensor("ident", [128, 128], BF16, kind="ExternalInput")
    out_d = nc.dram_tensor("out", [CC, 128, HW], BF16, kind="ExternalOutput")

    with tile.TileContext(nc) as tc, ExitStack() as ctx:
        cst = ctx.enter_context(tc.tile_pool(name="cst", bufs=1))
        dram = ctx.enter_context(tc.tile_pool(name="dram", bufs=1, space="DRAM"))
        actx = ExitStack()  # phase A/B scratch (closed before main loop)
        sm = actx.enter_context(tc.tile_pool(name="sm", bufs=9))
        pconv = actx.enter_context(tc.tile_pool(name="pconv", bufs=1, space="PSUM"))
        pt = actx.enter_context(tc.tile_pool(name="pt", bufs=2, space="PSUM"))
        xtp = actx.enter_context(tc.tile_pool(name="xtp", bufs=2))

        # ---- constants / image ----
        xsb = cst.tile([128, CC, HW], BF16)
        nc.sync.dma_start(out=xsb[:], in_=x_d.ap().transpose([1, 0, 2]))
        xp = cst.tile([128, CC, 34, 34], BF16)
        nc.vector.memset(xp[:], 0.0)
        for cc_ld in range(CC):
            nc.sync.dma_start(
                out=xp[:, cc_ld, 1:33, 1:33],
                in_=x_d.ap().transpose([1, 0, 2])[:, cc_ld].rearrange(
                    "p (y x) -> p y x", y=32
                ),
            )
        womsb = cst.tile([128, K, CC, 27], BF16)
        nc.sync.dma_start(out=womsb[:], in_=wom_d.ap().transpose([2, 0, 1, 3]))
        bom = cst.tile([27, 1], F32)
        nc.sync.dma_start(out=bom[:], in_=bom_d[:])
        gb = cst.tile([18, HW], F32)
        nc.sync.dma_start(out=gb[:], in_=gb_d[:])
        pb = cst.tile([128, CC], F32)
        nc.sync.dma_start(out=pb[:], in_=pb_d.ap().transpose([1, 0]))
        gam = cst.tile([128, CC], F32)
        nc.sync.dma_start(out=gam[:], in_=gam_d.ap().transpose([1, 0]))
        bet = cst.tile([128, CC], F32)
        nc.sync.dma_start(out=bet[:], in_=bet_d.ap().transpose([1, 0]))
        ident = cst.tile([128, 128], BF16)
        nc.sync.dma_start(out=ident[:], in_=id_d[:])

        # ---- offset/mask conv: psum27[oc, hw] over 54 (cc,k) matmuls ----
        psum27 = pconv.tile([27, HW], F32)
        for cc in range(CC):
            for k in range(K):
                ki, kj = k // 3, k % 3
                for h in range(2):
                    nc.tensor.matmul(
                        psum27[:, h * BLK : (h + 1) * BLK],
                        lhsT=womsb[:, k, cc, :],
                        rhs=xp[:, cc, ki + h * 16 : ki + h * 16 + 16, kj : kj + 32],
                        start=(cc == 0 and k == 0),
                        stop=(cc == CC - 1 and k == K - 1),
                    )

        # ---- pixel-major padded DRAM image xd[2688, 768] ----
        xd = dram.tile([XD_ROWS, C], BF16)
        zt = cst.tile([128, C], BF16)
        nc.vector.memset(zt[:], 0.0)
        for j in range(XD_ROWS // 128):
            eng = nc.sync if j % 2 == 0 else nc.scalar
            eng.dma_start(out=xd[j * 128 : (j + 1) * 128, :], in_=zt[:])
        for pb8 in range(8):
            xt = xtp.tile([128, C], BF16, tag="xt", name=f"xt{pb8}")
            for cc in range(CC):
                pst = pt.tile([128, 128], BF16, tag="pst", name=f"pst{pb8}_{cc}")
                nc.tensor.transpose(
                    pst[:],
                    xsb[:, cc, pb8 * 128 : (pb8 + 1) * 128],
                    ident[:],
                )
                nc.scalar.activation(
                    xt[:, cc * 128 : (cc + 1) * 128], pst[:], AF.Identity
                )
            for a in range(4):
                r0 = (PAD + pb8 * 4 + a) * PADC + PAD
                nc.sync.dma_start(
                    out=xd[r0 : r0 + 32, :], in_=xt[a * 32 : (a + 1) * 32, :]
                )

        # ---- small-tensor math ----
        # psum rows: dy taps 0-8, dx taps 9-17, mask 18-26
        omx = sm.tile([27, HW], F32, tag="s4")
        nc.scalar.activation(omx[:], psum27[:], AF.Identity, bias=bom[:])
        doff = sm.tile([18, HW], F32, tag="s4")
        nc.vector.tensor_scalar(doff[:], omx[0:18, :], 8.0, -8.0, OP.min, OP.max)
        s16 = sm.tile([18, HW], F32, tag="s4")
        nc.vector.tensor_tensor(s16[:], doff[:], gb[:], OP.add)
        i32 = sm.tile([18, HW], I32, tag="s4")
        nc.vector.tensor_copy(i32[:], s16[:])
        fint = sm.tile([18, HW], F32, tag="s4")
        nc.vector.tensor_copy(fint[:], i32[:])
        corr = sm.tile([18, HW], F32, tag="s4")
        nc.vector.tensor_tensor(corr[:], fint[:], s16[:], OP.is_gt)
        ffc = sm.tile([18, HW], F32, tag="s4")
        nc.vector.tensor_tensor(ffc[:], fint[:], corr[:], OP.subtract)
        frac = sm.tile([18, HW], F32, tag="s4")
        nc.vector.tensor_tensor(frac[:], s16[:], ffc[:], OP.subtract)
        u1 = sm.tile([18, HW], F32, tag="s4")
        nc.vector.tensor_scalar(u1[:], frac[:], -1.0, 1.0, OP.mult, OP.add)
        # extract x-role rows to partition-base-0 tiles (engine APs must
        # start at 32-aligned partitions; DMA re-bases them)
        frx = sm.tile([9, HW], F32, tag="s4")
        nc.scalar.dma_start(out=frx[:], in_=frac[9:18, :])
        u1x = sm.tile([9, HW], F32, tag="s4")
        nc.scalar.dma_start(out=u1x[:], in_=u1[9:18, :])
        ffx = sm.tile([9, HW], F32, tag="s4")
        nc.scalar.dma_start(out=ffx[:], in_=ffc[9:18, :])
        omm = sm.tile([9, HW], F32, tag="s4")
        nc.scalar.dma_start(out=omm[:], in_=omx[18:27, :])
        m2 = sm.tile([9, HW], F32, tag="s4")
        nc.scalar.activation(m2[:], omm[:], AF.Sigmoid)
        # gather index first (gates the gathers): idx = ffc_y*52 + ffc_x
        idxf = sm.tile([9, HW], F32, tag="s4")
        nc.vector.scalar_tensor_tensor(
            idxf[:], ffc[0:9, :], float(PADC), ffx[:], OP.mult, OP.add
        )
        idx16 = sm.tile([9, HW], I16, tag="s4")
        nc.vector.tensor_copy(idx16[:], idxf[:])

        # wrapped layout: idxw[p, s] = flat[16*s + p%16], replicated x8;
        # column blocks of 32 per (b2, k).
        idxw = cst.tile([128, NB * K * 32], I16)
        for b2 in range(NB):
            for k in range(K):
                eng1 = nc.sync if k % 2 == 0 else nc.scalar
                eng2 = nc.scalar if k % 2 == 0 else nc.sync
                t1w = sm.tile([32, 16], I16, tag="t1w", name="t1w", bufs=4)
                eng1.dma_start(
                    out=t1w[:],
                    in_=idx16[k : k + 1, b2 * BLK : (b2 + 1) * BLK].rearrange(
                        "o (h r) -> o h r", h=32, r=16
                    ),
                )
                t2w = sm.tile([32, 128], I16, tag="t2w", name="t2w", bufs=4)
                eng2.dma_start(
                    out=t2w[:].rearrange("h (g r) -> h g r", g=8, r=16),
                    in_=t1w[:].unsqueeze(1).broadcast_to((32, 8, 16)),
                )
                cb = (b2 * K + k) * 32
                nc.sync.dma_start(
                    out=idxw[:, cb : cb + 32], in_=t2w[:], transpose=True
                )

        wA = sm.tile([9, HW], F32, tag="s4")
        nc.vector.scalar_tensor_tensor(wA[:], m2[:], 2.0, u1[0:9, :], OP.mult, OP.mult)
        wB = sm.tile([9, HW], F32, tag="s4")
        nc.vector.scalar_tensor_tensor(wB[:], m2[:], 2.0, frac[0:9, :], OP.mult, OP.mult)
        a4 = sm.tile([9, 4, HW], BF16, tag="a4")
        nc.vector.tensor_tensor(a4[:, 0, :], wA[:], u1x[:], OP.mult)
        nc.vector.tensor_tensor(a4[:, 1, :], wA[:], frx[:], OP.mult)
        nc.vector.tensor_tensor(a4[:, 2, :], wB[:], u1x[:], OP.mult)
        nc.vector.tensor_tensor(a4[:, 3, :], wB[:], frx[:], OP.mult)
        mwd = dram.tile([K, 4, HW], BF16)
        nc.scalar.dma_start(out=mwd[:], in_=a4[:])

        actx.close()

        # gather source views: rows of 1536 elems (2 pixels) at stride 768
        srcT = xd[:].copy()
        srcT.ap = bass_rust.VecI64Pair([(C, VIEW_ROWS), (1, 2 * C)])
        srcB = xd[PADC:, :].copy()
        srcB.ap = bass_rust.VecI64Pair([(C, VIEW_ROWS), (1, 2 * C)])

        mctx = ExitStack()
        vpool = mctx.enter_context(tc.tile_pool(name="vp", bufs=2))
        rpool = mctx.enter_context(tc.tile_pool(name="rp", bufs=2))
        mpool = mctx.enter_context(tc.tile_pool(name="mp", bufs=2))
        wpool = mctx.enter_context(tc.tile_pool(name="wp", bufs=2))
        pmain = mctx.enter_context(tc.tile_pool(name="pmain", bufs=1, space="PSUM"))

        ysb = cst.tile([128, CC, HW], BF16)
        stats = cst.tile([128, 4 * CC], F32)  # [S_b0|S_b1|Q_b0|Q_b1]
        sqscr = cst.tile([128, BLK], F32)

        # ---- main loop: gather / combine / matmul ----
        for b2 in range(NB):
            psums = [
                pmain.tile([128, BLK], F32, tag=f"ps{o}", name=f"psum_b{b2}_o{o}")
                for o in range(CC)
            ]
            for kg0, kg1 in KGS:
                wt = wpool.tile([128, 3, CC, C], BF16, tag="wt", name="wt")
                nc.sync.dma_start(
                    out=wt[:], in_=wproj_d[kg0:kg1].transpose([2, 0, 1, 3])
                )
                mbT = mpool.tile([128, 3, 2, BLK], BF16, tag="mT", name="mbT")
                mbB = mpool.tile([128, 3, 2, BLK], BF16, tag="mB", name="mbB")
                for j in range(2):
                    nc.scalar.dma_start(
                        out=mbT[:, :, j, :],
                        in_=mwd[kg0:kg1, j, b2 * BLK : (b2 + 1) * BLK]
                        .unsqueeze(0)
                        .broadcast_to((128, 3, BLK)),
                    )
                    nc.scalar.dma_start(
                        out=mbB[:, :, j, :],
                        in_=mwd[kg0:kg1, 2 + j, b2 * BLK : (b2 + 1) * BLK]
                        .unsqueeze(0)
                        .broadcast_to((128, 3, BLK)),
                    )
                for k in range(kg0, kg1):
                    cb = (b2 * K + k) * 32
                    vT = vpool.tile([128, 2, CC, BLK], BF16, tag="vT", name="vT")
                    nc.gpsimd.dma_gather(
                        vT[:].rearrange("p a c n -> p (a c) n"),
                        srcT, idxw[:, cb : cb + 32],
                        BLK, BLK, 2 * C, elem_step=C, transpose=True,
                    )
                    vB = vpool.tile([128, 2, CC, BLK], BF16, tag="vB", name="vB")
                    nc.gpsimd.dma_gather(
                        vB[:].rearrange("p a c n -> p (a c) n"),
                        srcB, idxw[:, cb : cb + 32],
                        BLK, BLK, 2 * C, elem_step=C, transpose=True,
                    )
                    mTk = (
                        mbT[:, k - kg0]
                        .unsqueeze(2)
                        .broadcast_to((128, 2, CC, BLK))
                    )
                    mBk = (
                        mbB[:, k - kg0]
                        .unsqueeze(2)
                        .broadcast_to((128, 2, CC, BLK))
                    )
                    nc.vector.tensor_tensor(vT[:], vT[:], mTk, OP.mult)
                    nc.vector.tensor_tensor(vB[:], vB[:], mBk, OP.mult)
                    R = rpool.tile([128, CC, BLK], BF16, tag="R", name="R")
                    nc.vector.tensor_tensor(R[:], vT[:, 0], vT[:, 1], OP.add)
                    tmpB = rpool.tile([128, CC, BLK], BF16, tag="tB", name="tB")
                    nc.vector.tensor_tensor(tmpB[:], vB[:, 0], vB[:, 1], OP.add)
                    nc.vector.tensor_tensor(R[:], R[:], tmpB[:], OP.add)
                    for cc in range(CC):
                        for o in range(CC):
                            nc.tensor.matmul(
                                psums[o][:],
                                lhsT=wt[:, k - kg0, cc, o * 128 : (o + 1) * 128],
                                rhs=R[:, cc, :],
                                start=(k == 0 and cc == 0),
                                stop=(k == K - 1 and cc == CC - 1),
                            )
            for o in range(CC):
                nc.scalar.activation(
                    ysb[:, o, b2 * BLK : (b2 + 1) * BLK],
                    psums[o][:],
                    AF.Identity,
                    bias=pb[:, o : o + 1],
                    accum_out=stats[:, b2 * CC + o : b2 * CC + o + 1],
                )
                nc.scalar.activation(
                    sqscr[:],
                    ysb[:, o, b2 * BLK : (b2 + 1) * BLK],
                    AF.Square,
                    accum_out=stats[:, (2 + b2) * CC + o : (2 + b2) * CC + o + 1],
                )

        mctx.close()
        opool = ctx.enter_context(tc.tile_pool(name="op", bufs=2))
        smf = ctx.enter_context(tc.tile_pool(name="smf", bufs=4))

        # ---- SyncBN stats all-reduce ----
        ssum = smf.tile([128, 2 * CC], F32, tag="f")
        nc.vector.tensor_tensor(
            ssum[:, 0:CC], stats[:, 0:CC], stats[:, CC : 2 * CC], OP.add
        )
        nc.vector.tensor_tensor(
            ssum[:, CC : 2 * CC],
            stats[:, 2 * CC : 3 * CC],
            stats[:, 3 * CC : 4 * CC],
            OP.add,
        )
        statloc = dram.tile([128, 2 * CC], F32)
        statglob = dram.tile([128, 2 * CC], F32, addr_space="Shared")
        nc.sync.dma_start(out=statloc[:], in_=ssum[:])
        if mock_cc:
            nc.sync.dma_start(out=statglob[:], in_=statloc[:])
        else:
            nc.gpsimd.collective_compute(
                "AllReduce",
                OP.add,
                replica_groups=[list(range(N_CORES))],
                ins=[statloc[:]],
                outs=[statglob[:]],
            )
        gst = smf.tile([128, 2 * CC], F32, tag="f")
        nc.sync.dma_start(out=gst[:], in_=statglob[:])

        inv_n = 1.0 / (B * HW)
        mean = smf.tile([128, CC], F32, tag="f")
        nc.vector.tensor_scalar(mean[:], gst[:, 0:CC], inv_n, None, OP.mult)
        ex2 = smf.tile([128, CC], F32, tag="f")
        nc.vector.tensor_scalar(ex2[:], gst[:, CC : 2 * CC], inv_n, None, OP.mult)
        var = smf.tile([128, CC], F32, tag="f")
        nc.vector.scalar_tensor_tensor(var[:], mean[:], 1.0, mean[:], OP.mult, OP.mult)
        nc.vector.tensor_tensor(var[:], ex2[:], var[:], OP.subtract)
        epst = smf.tile([128, 1], F32, tag="f")
        nc.vector.memset(epst[:], EPS)
        std = smf.tile([128, CC], F32, tag="f")
        nc.scalar.activation(std[:], var[:], AF.Sqrt, bias=epst[:])
        inv = smf.tile([128, CC], F32, tag="f")
        nc.vector.reciprocal(inv[:], std[:])
        scl = smf.tile([128, CC], F32, tag="f")
        nc.vector.tensor_tensor(scl[:], gam[:], inv[:], OP.mult)
        sft = smf.tile([128, CC], F32, tag="f")
        nc.vector.tensor_tensor(sft[:], mean[:], scl[:], OP.mult)
        nc.vector.tensor_tensor(sft[:], bet[:], sft[:], OP.subtract)

        # ---- normalize + erf-GELU + residual ----
        for cc in range(CC):
            for hb in range(NB):
                hs = slice(hb * BLK, (hb + 1) * BLK)
                yn = opool.tile([128, BLK], F32, tag="yn", name="yn")
                nc.vector.tensor_scalar(
                    yn[:],
                    ysb[:, cc, hs],
                    scl[:, cc : cc + 1],
                    sft[:, cc : cc + 1],
                    OP.mult,
                    OP.add,
                )
                gel = opool.tile([128, BLK], F32, tag="gel", name="gel")
                nc.scalar.activation(gel[:], yn[:], AF.Gelu)
                ob = opool.tile([128, BLK], BF16, tag="ob", name="ob")
                nc.vector.tensor_tensor(
                    ob[:], gel[:], xp[:, cc, 1 + hb * 16 : 1 + hb * 16 + 16, 1:33],
                    OP.add,
                )
                nc.scalar.dma_start(out=out_d[cc][:, hs], in_=ob[:])

    nc.compile()
    return nc


def _host_prep(inputs):
    x = np.asarray(inputs["x"], np.float32)
    proj_w = np.asarray(inputs["proj_w"], np.float32)
    proj_b = np.asarray(inputs["proj_b"], np.float32)
    offset_w = np.asarray(inputs["offset_w"], np.float32)
    offset_b = np.asarray(inputs["offset_b"], np.float32)
    mask_w = np.asarray(inputs["mask_w"], np.float32)
    mask_b = np.asarray(inputs["mask_b"], np.float32)
    gamma = np.asarray(inputs["gamma"], np.float32)
    beta = np.asarray(inputs["beta"], np.float32)

    bf = ml_dtypes.bfloat16
    x16 = x.reshape(B, CC, 128, HW).astype(bf)

    # proj weights -> [k, cc, c128, o]
    wproj = (
        proj_w.reshape(C, C, K)
        .transpose(2, 1, 0)
        .reshape(K, CC, 128, C)
        .astype(bf)
    )
    # dy taps rows 0-8, dx rows 9-17, mask rows 18-26
    ow = offset_w.reshape(K, 2, C, K)
    om_w = np.concatenate([ow[:, 0], ow[:, 1], mask_w.reshape(K, C, K)], axis=0)
    wom = om_w.transpose(2, 1, 0).reshape(K, CC, 128, 27).astype(bf)
    ob = offset_b.reshape(K, 2)
    bom = np.concatenate([ob[:, 0], ob[:, 1], mask_b]).reshape(27, 1).astype(np.float32)

    hh, ww = np.meshgrid(np.arange(H), np.arange(W), indexing="ij")
    gb = np.zeros((18, HW), np.float32)
    for k in range(K):
        ki, kj = k // 3, k % 3
        gb[k] = (hh + ki - 1 + PAD).reshape(-1)
        gb[9 + k] = (ww + kj - 1 + PAD).reshape(-1)

    pb = proj_b.reshape(CC, 128).astype(np.float32)
    gam2 = gamma.reshape(CC, 128).astype(np.float32)
    bet2 = beta.reshape(CC, 128).astype(np.float32)
    ident = np.eye(128, dtype=bf)

    shared = {
        "wproj": wproj,
        "wom": wom,
        "bom": bom,
        "gb16": gb,
        "pbias": pb,
        "gamma2": gam2,
        "beta2": bet2,
        "ident": ident,
    }
    in_maps = []
    for b in range(B):
        m = dict(shared)
        m["x16"] = x16[b]
        in_maps.append(m)
    return in_maps


def kernel(**inputs):
    if "nc" not in _CACHE:
        _CACHE["nc"] = _build_program()
    nc = _CACHE["nc"]
    in_maps = _host_prep(inputs)
    res = run_bass_kernel_spmd(nc, in_maps, list(range(N_CORES)))
    out = np.stack(
        [r["out"].astype(np.float32).reshape(C, H, W) for r in res.results]
    )
    return out


if __name__ == "__main__":
    nc = _build_program()
    print("program built OK;", len(nc.m.functions[0].blocks), "blocks")


# revision 11
# speedup vs baseline: 5.5317x; 1.0736x over previous
"""Trainium2 Bass kernel for DeformableConv2 block (offset/mask conv ->
modulated deformable conv -> SyncBN -> GELU -> residual).

Sharding: data-parallel over batch B=8 across 8 cores (1 image/core),
weights replicated, BN statistics all-reduced (SyncBatchNorm).

Pipeline per core (image b):
  1. x loaded once as [128, 6, 34, 34] zero-padded bf16 (xp). PE
     transposes build a pixel-major padded DRAM copy xd[2688, 768]
     (rows = padded pixels, 768 channels contiguous per row).
  2. offset/mask 3x3 conv as 54 accumulated bf16 matmuls from strided
     views of xp, fp32 PSUM [27, 1024].
  3. Small-tensor math produces: int16 gather indices (top-left padded
     pixel of each bilinear 2x2 patch) in the 16-partition-wrapped
     layout, and 4 mask-folded bilinear weights (a00,a01,a10,a11) in
     DRAM for per-block broadcast.
  4. HWDGE dma_gather(transpose=True, elem=1536, step=768): one call
     per (half-image, tap, top/bottom) pulls 512 horizontal pixel
     pairs across all 768 channels, transposed to channel-partition
     layout [128, 12, 512]. This runs on the DMA engines at HBM
     bandwidth instead of the Q7 cores (the old ap_gather bottleneck).
  5. DVE combine (5 big ops per (half, tap)) folds the 4 bilinear
     weights + mask: R[128, 6, 512] bf16.
  6. PE contracts wproj[o,c,k] against R: 36 matmuls per (half, tap),
     fp32 PSUM, 6 banks (one per 128-wide output-channel chunk).
  7. Per-channel sum/sumsq via ACT accum_out; [128,12] AllReduce
     across the 8 cores; normalize + erf-GELU + residual (from xp).
"""

import sys

sys.path.insert(0, "/opt/trn_rl_repo")

from contextlib import ExitStack

import ml_dtypes
import numpy as np

import bass_rust
import concourse.bacc as bacc
import concourse.bass as bass
import concourse.tile as tile
from concourse import mybir
from concourse.bass_utils import run_bass_kernel_spmd

F32 = mybir.dt.float32
BF16 = mybir.dt.bfloat16
I16 = mybir.dt.int16
I32 = mybir.dt.int32
AF = mybir.ActivationFunctionType
OP = mybir.AluOpType

B, C, H, W = 8, 768, 32, 32
CC = C // 128            # 6 channel chunks
HW = H * W               # 1024
K = 9                    # 3x3 taps
PAD = 9                  # sample coords in [-9, 40] -> padded rows [0, 50]
PADC = 52                # padded row stride (51 cols + 1 for x+1 pair)
XD_ROWS = 2688           # 21*128 (>= 51*52 + slack for +52 B-view)
VIEW_ROWS = 2600         # gather view row count (> max idx 2597)
BLK = 512                # hw block (matmul moving dim / gather call size)
NB = HW // BLK           # 2
KGS = [(0, 3), (3, 6), (6, 9)]
EPS = 1e-5
N_CORES = 8

_CACHE = {}


def _build_program(mock_cc=False):
    nc = bacc.Bacc("TRN2", target_bir_lowering=False)

    # ---- DRAM I/O ----
    x_d = nc.dram_tensor("x16", [CC, 128, HW], BF16, kind="ExternalInput")
    wproj_d = nc.dram_tensor("wproj", [3, 128, 3 * CC * C], BF16, kind="ExternalInput")
    wom_d = nc.dram_tensor("wom", [K, CC, 128, 27], BF16, kind="ExternalInput")
    bom_d = nc.dram_tensor("bom", [27, 1], F32, kind="ExternalInput")
    gb_d = nc.dram_tensor("gb16", [18, HW], F32, kind="ExternalInput")
    pb_d = nc.dram_tensor("pbias", [CC, 128], F32, kind="ExternalInput")
    gam_d = nc.dram_tensor("gamma2", [CC, 128], F32, kind="ExternalInput")
    bet_d = nc.dram_tensor("beta2", [CC, 128], F32, kind="ExternalInput")
    id_d = nc.dram_tensor("ident", [128, 128], BF16, kind="ExternalInput")
    out_d = nc.dram_tensor("out", [CC, 128, HW], BF16, kind="ExternalOutput")

    with tile.TileContext(nc) as tc, ExitStack() as ctx:
        cst = ctx.enter_context(tc.tile_pool(name="cst", bufs=1))
        dram = ctx.enter_context(tc.tile_pool(name="dram", bufs=1, space="DRAM"))
        actx = ExitStack()  # phase A/B scratch (closed before main loop)
        sm = actx.enter_context(tc.tile_pool(name="sm", bufs=9))
        pconv = actx.enter_context(tc.tile_pool(name="pconv", bufs=1, space="PSUM"))
        pt = actx.enter_context(tc.tile_pool(name="pt", bufs=2, space="PSUM"))
        xtp = actx.enter_context(tc.tile_pool(name="xtp", bufs=2))

        # ---- constants / image ----
        xsb = cst.tile([128, CC, HW], BF16)
        nc.sync.dma_start(out=xsb[:], in_=x_d.ap().transpose([1, 0, 2]))
        xp = cst.tile([128, CC, 34, 34], BF16)
        nc.vector.memset(xp[:], 0.0)
        nc.vector.tensor_copy(
            xp[:, :, 1:33, 1:33],
            xsb[:].rearrange("p c (y x) -> p c y x", y=32),
        )
        womsb = cst.tile([128, K, CC, 27], BF16)
        nc.sync.dma_start(out=womsb[:], in_=wom_d.ap().transpose([2, 0, 1, 3]))
        bom = cst.tile([27, 1], F32)
        nc.sync.dma_start(out=bom[:], in_=bom_d[:])
        gb = cst.tile([18, HW], F32)
        nc.sync.dma_start(out=gb[:], in_=gb_d[:])
        pb = cst.tile([128, CC], F32)
        nc.sync.dma_start(out=pb[:], in_=pb_d.ap().transpose([1, 0]))
        gam = cst.tile([128, CC], F32)
        nc.sync.dma_start(out=gam[:], in_=gam_d.ap().transpose([1, 0]))
        bet = cst.tile([128, CC], F32)
        nc.sync.dma_start(out=bet[:], in_=bet_d.ap().transpose([1, 0]))
        ident = cst.tile([128, 128], BF16)
        nc.sync.dma_start(out=ident[:], in_=id_d[:])

        # ---- offset/mask conv: psum27[oc, hw] over 54 (cc,k) matmuls ----
        psum27 = pconv.tile([27, HW], F32)
        for cc in range(CC):
            for k in range(K):
                ki, kj = k // 3, k % 3
                for h in range(2):
                    nc.tensor.matmul(
                        psum27[:, h * BLK : (h + 1) * BLK],
                        lhsT=womsb[:, k, cc, :],
                        rhs=xp[:, cc, ki + h * 16 : ki + h * 16 + 16, kj : kj + 32],
                        start=(cc == 0 and k == 0),
                        stop=(cc == CC - 1 and k == K - 1),
                    )

        # ---- pixel-major padded DRAM image xd[2688, 768] ----
        xd = dram.tile([XD_ROWS, C], BF16)
        zt = cst.tile([128, 4 * C], BF16)
        nc.vector.memset(zt[:], 0.0)
        for j in range(XD_ROWS // 512):
            eng = nc.sync if j % 2 == 0 else nc.scalar
            eng.dma_start(
                out=xd[j * 512 : (j + 1) * 512, :].rearrange(
                    "(p r) c -> p r c", p=128, r=4
                ),
                in_=zt[:].rearrange("p (r c) -> p r c", r=4),
            )
        nc.scalar.dma_start(
            out=xd[2560:2688, :], in_=zt[:, 0:C]
        )
        for pb8 in range(8):
            xt = xtp.tile([128, C], BF16, tag="xt", name=f"xt{pb8}")
            for cc in range(CC):
                pst = pt.tile([128, 128], BF16, tag="pst", name=f"pst{pb8}_{cc}")
                nc.tensor.transpose(
                    pst[:],
                    xsb[:, cc, pb8 * 128 : (pb8 + 1) * 128],
                    ident[:],
                )
                nc.scalar.activation(
                    xt[:, cc * 128 : (cc + 1) * 128], pst[:], AF.Identity
                )
            for a in range(4):
                r0 = (PAD + pb8 * 4 + a) * PADC + PAD
                nc.sync.dma_start(
                    out=xd[r0 : r0 + 32, :], in_=xt[a * 32 : (a + 1) * 32, :]
                )

        # ---- small-tensor math ----
        # psum rows: dy taps 0-8, dx taps 9-17, mask 18-26
        omx = sm.tile([27, HW], F32, tag="s4")
        nc.scalar.activation(omx[:], psum27[:], AF.Identity, bias=bom[:])
        doff = sm.tile([18, HW], F32, tag="s4")
        nc.vector.tensor_scalar(doff[:], omx[0:18, :], 8.0, -8.0, OP.min, OP.max)
        s16 = sm.tile([18, HW], F32, tag="s4")
        nc.vector.tensor_tensor(s16[:], doff[:], gb[:], OP.add)
        i32 = sm.tile([18, HW], I32, tag="s4")
        nc.vector.tensor_copy(i32[:], s16[:])
        fint = sm.tile([18, HW], F32, tag="s4")
        nc.vector.tensor_copy(fint[:], i32[:])
        corr = sm.tile([18, HW], F32, tag="s4")
        nc.vector.tensor_tensor(corr[:], fint[:], s16[:], OP.is_gt)
        ffc = sm.tile([18, HW], F32, tag="s4")
        nc.vector.tensor_tensor(ffc[:], fint[:], corr[:], OP.subtract)
        frac = sm.tile([18, HW], F32, tag="s4")
        nc.vector.tensor_tensor(frac[:], s16[:], ffc[:], OP.subtract)
        u1 = sm.tile([18, HW], F32, tag="s4")
        nc.vector.tensor_scalar(u1[:], frac[:], -1.0, 1.0, OP.mult, OP.add)
        # extract x-role rows to partition-base-0 tiles (engine APs must
        # start at 32-aligned partitions; DMA re-bases them)
        frx = sm.tile([9, HW], F32, tag="s4")
        nc.scalar.dma_start(out=frx[:], in_=frac[9:18, :])
        u1x = sm.tile([9, HW], F32, tag="s4")
        nc.scalar.dma_start(out=u1x[:], in_=u1[9:18, :])
        ffx = sm.tile([9, HW], F32, tag="s4")
        nc.scalar.dma_start(out=ffx[:], in_=ffc[9:18, :])
        omm = sm.tile([9, HW], F32, tag="s4")
        nc.scalar.dma_start(out=omm[:], in_=omx[18:27, :])
        m2 = sm.tile([9, HW], F32, tag="s4")
        nc.scalar.activation(m2[:], omm[:], AF.Sigmoid)
        # gather index first (gates the gathers): idx = ffc_y*52 + ffc_x
        idxf = sm.tile([9, HW], F32, tag="s4")
        nc.vector.scalar_tensor_tensor(
            idxf[:], ffc[0:9, :], float(PADC), ffx[:], OP.mult, OP.add
        )
        idx16 = sm.tile([9, HW], I16, tag="s4")
        nc.vector.tensor_copy(idx16[:], idxf[:])

        # wrapped layout: idxw[p, s] = flat[16*s + p%16], replicated x8;
        # column blocks of 32 per (b2, k).
        idxw = cst.tile([128, NB * K * 32], I16)
        for b2 in range(NB):
            for k in range(K):
                eng1 = nc.sync if k % 2 == 0 else nc.scalar
                eng2 = nc.scalar if k % 2 == 0 else nc.sync
                t1w = sm.tile([32, 16], I16, tag="t1w", name="t1w", bufs=4)
                eng1.dma_start(
                    out=t1w[:],
                    in_=idx16[k : k + 1, b2 * BLK : (b2 + 1) * BLK].rearrange(
                        "o (h r) -> o h r", h=32, r=16
                    ),
                )
                t2w = sm.tile([32, 128], I16, tag="t2w", name="t2w", bufs=4)
                nc.vector.tensor_copy(
                    t2w[:].rearrange("h (g r) -> h g r", g=8, r=16),
                    t1w[:].unsqueeze(1).broadcast_to((32, 8, 16)),
                )
                cb = (b2 * K + k) * 32
                nc.sync.dma_start(
                    out=idxw[:, cb : cb + 32], in_=t2w[:], transpose=True
                )

        wA = sm.tile([9, HW], F32, tag="s4")
        nc.vector.scalar_tensor_tensor(wA[:], m2[:], 2.0, u1[0:9, :], OP.mult, OP.mult)
        wB = sm.tile([9, HW], F32, tag="s4")
        nc.vector.scalar_tensor_tensor(wB[:], m2[:], 2.0, frac[0:9, :], OP.mult, OP.mult)
        a4 = sm.tile([9, 4, HW], BF16, tag="a4")
        nc.vector.tensor_tensor(a4[:, 0, :], wA[:], u1x[:], OP.mult)
        nc.vector.tensor_tensor(a4[:, 1, :], wA[:], frx[:], OP.mult)
        nc.vector.tensor_tensor(a4[:, 2, :], wB[:], u1x[:], OP.mult)
        nc.vector.tensor_tensor(a4[:, 3, :], wB[:], frx[:], OP.mult)
        mwd = dram.tile([K, 4, HW], BF16)
        nc.scalar.dma_start(out=mwd[:], in_=a4[:])

        actx.close()

        # gather source views: rows of 1536 elems (2 pixels) at stride 768
        srcT = xd[:].copy()
        srcT.ap = bass_rust.VecI64Pair([(C, VIEW_ROWS), (1, 2 * C)])
        srcB = xd[PADC:, :].copy()
        srcB.ap = bass_rust.VecI64Pair([(C, VIEW_ROWS), (1, 2 * C)])

        mctx = ExitStack()
        vpool = mctx.enter_context(tc.tile_pool(name="vp", bufs=2))
        rpool = mctx.enter_context(tc.tile_pool(name="rp", bufs=2))
        mpool = mctx.enter_context(tc.tile_pool(name="mp", bufs=2))
        wpool = mctx.enter_context(tc.tile_pool(name="wp", bufs=2))
        pmain = mctx.enter_context(tc.tile_pool(name="pmain", bufs=1, space="PSUM"))

        ysb = cst.tile([128, CC, HW], BF16)
        stats = cst.tile([128, 4 * CC], F32)  # [S_b0|S_b1|Q_b0|Q_b1]
        sqscr = cst.tile([128, BLK], F32)

        # ---- main loop: gather / combine / matmul ----
        for b2 in range(NB):
            psums = [
                pmain.tile([128, BLK], F32, tag=f"ps{o}", name=f"psum_b{b2}_o{o}")
                for o in range(CC)
            ]
            for kg0, kg1 in KGS:
                wt = wpool.tile([128, 3, CC, C], BF16, tag="wt", name="wt")
                nc.sync.dma_start(
                    out=wt[:].rearrange("p a c o -> p (a c o)"),
                    in_=wproj_d[kg0 // 3],
                )
                mbT = mpool.tile([128, 2, 3, BLK], BF16, tag="mT", name="mbT")
                mbB = mpool.tile([128, 2, 3, BLK], BF16, tag="mB", name="mbB")
                for j in range(2):
                    nc.scalar.dma_start(
                        out=mbT[:, j],
                        in_=mwd[kg0:kg1, j, b2 * BLK : (b2 + 1) * BLK]
                        .unsqueeze(0)
                        .broadcast_to((128, 3, BLK)),
                    )
                    nc.scalar.dma_start(
                        out=mbB[:, j],
                        in_=mwd[kg0:kg1, 2 + j, b2 * BLK : (b2 + 1) * BLK]
                        .unsqueeze(0)
                        .broadcast_to((128, 3, BLK)),
                    )
                for k in range(kg0, kg1):
                    cb = (b2 * K + k) * 32
                    vT = vpool.tile([128, 2, CC, BLK], BF16, tag="vT", name="vT")
                    nc.gpsimd.dma_gather(
                        vT[:].rearrange("p a c n -> p (a c) n"),
                        srcT, idxw[:, cb : cb + 32],
                        BLK, BLK, 2 * C, elem_step=C, transpose=True,
                    )
                    vB = vpool.tile([128, 2, CC, BLK], BF16, tag="vB", name="vB")
                    nc.gpsimd.dma_gather(
                        vB[:].rearrange("p a c n -> p (a c) n"),
                        srcB, idxw[:, cb : cb + 32],
                        BLK, BLK, 2 * C, elem_step=C, transpose=True,
                    )
                    mTk = (
                        mbT[:, :, k - kg0, :]
                        .unsqueeze(2)
                        .broadcast_to((128, 2, CC, BLK))
                    )
                    mBk = (
                        mbB[:, :, k - kg0, :]
                        .unsqueeze(2)
                        .broadcast_to((128, 2, CC, BLK))
                    )
                    nc.vector.tensor_tensor(vT[:], vT[:], mTk, OP.mult)
                    nc.vector.tensor_tensor(vB[:], vB[:], mBk, OP.mult)
                    R = rpool.tile([128, CC, BLK], BF16, tag="R", name="R")
                    nc.vector.tensor_tensor(R[:], vT[:, 0], vT[:, 1], OP.add)
                    tmpB = rpool.tile([128, CC, BLK], BF16, tag="tB", name="tB")
                    nc.vector.tensor_tensor(tmpB[:], vB[:, 0], vB[:, 1], OP.add)
                    nc.vector.tensor_tensor(R[:], R[:], tmpB[:], OP.add)
                    for cc in range(CC):
                        for o in range(CC):
                            nc.tensor.matmul(
                                psums[o][:],
                                lhsT=wt[:, k - kg0, cc, o * 128 : (o + 1) * 128],
                                rhs=R[:, cc, :],
                                start=(k == 0 and cc == 0),
                                stop=(k == K - 1 and cc == CC - 1),
                            )
            for o in range(CC):
                nc.scalar.activation(
                    ysb[:, o, b2 * BLK : (b2 + 1) * BLK],
                    psums[o][:],
                    AF.Identity,
                    bias=pb[:, o : o + 1],
                    accum_out=stats[:, b2 * CC + o : b2 * CC + o + 1],
                )
                nc.scalar.activation(
                    sqscr[:],
                    ysb[:, o, b2 * BLK : (b2 + 1) * BLK],
                    AF.Square,
                    accum_out=stats[:, (2 + b2) * CC + o : (2 + b2) * CC + o + 1],
                )

        mctx.close()
        opool = ctx.enter_context(tc.tile_pool(name="op", bufs=2))
        smf = ctx.enter_context(tc.tile_pool(name="smf", bufs=4))

        # ---- SyncBN stats all-reduce ----
        ssum = smf.tile([128, 2 * CC], F32, tag="f")
        nc.vector.tensor_tensor(
            ssum[:, 0:CC], stats[:, 0:CC], stats[:, CC : 2 * CC], OP.add
        )
        nc.vector.tensor_tensor(
            ssum[:, CC : 2 * CC],
            stats[:, 2 * CC : 3 * CC],
            stats[:, 3 * CC : 4 * CC],
            OP.add,
        )
        statloc = dram.tile([128, 2 * CC], F32)
        statglob = dram.tile([128, 2 * CC], F32, addr_space="Shared")
        nc.sync.dma_start(out=statloc[:], in_=ssum[:])
        if mock_cc:
            nc.sync.dma_start(out=statglob[:], in_=statloc[:])
        else:
            nc.gpsimd.collective_compute(
                "AllReduce",
                OP.add,
                replica_groups=[list(range(N_CORES))],
                ins=[statloc[:]],
                outs=[statglob[:]],
            )
        gst = smf.tile([128, 2 * CC], F32, tag="f")
        nc.sync.dma_start(out=gst[:], in_=statglob[:])

        inv_n = 1.0 / (B * HW)
        mean = smf.tile([128, CC], F32, tag="f")
        nc.vector.tensor_scalar(mean[:], gst[:, 0:CC], inv_n, None, OP.mult)
        ex2 = smf.tile([128, CC], F32, tag="f")
        nc.vector.tensor_scalar(ex2[:], gst[:, CC : 2 * CC], inv_n, None, OP.mult)
        var = smf.tile([128, CC], F32, tag="f")
        nc.vector.scalar_tensor_tensor(var[:], mean[:], 1.0, mean[:], OP.mult, OP.mult)
        nc.vector.tensor_tensor(var[:], ex2[:], var[:], OP.subtract)
        epst = smf.tile([128, 1], F32, tag="f")
        nc.vector.memset(epst[:], EPS)
        std = smf.tile([128, CC], F32, tag="f")
        nc.scalar.activation(std[:], var[:], AF.Sqrt, bias=epst[:])
        inv = smf.tile([128, CC], F32, tag="f")
        nc.vector.reciprocal(inv[:], std[:])
        scl = smf.tile([128, CC], F32, tag="f")
        nc.vector.tensor_tensor(scl[:], gam[:], inv[:], OP.mult)
        sft = smf.tile([128, CC], F32, tag="f")
        nc.vector.tensor_tensor(sft[:], mean[:], scl[:], OP.mult)
        nc.vector.tensor_tensor(sft[:], bet[:], sft[:], OP.subtract)

        # ---- normalize + erf-GELU + residual ----
        for cc in range(CC):
            for hb in range(NB):
                hs = slice(hb * BLK, (hb + 1) * BLK)
                yn = opool.tile([128, BLK], F32, tag="yn", name="yn")
                nc.vector.tensor_scalar(
                    yn[:],
                    ysb[:, cc, hs],
                    scl[:, cc : cc + 1],
                    sft[:, cc : cc + 1],
                    OP.mult,
                    OP.add,
                )
                gel = opool.tile([128, BLK], F32, tag="gel", name="gel")
                nc.scalar.activation(gel[:], yn[:], AF.Gelu)
                ob = opool.tile([128, BLK], BF16, tag="ob", name="ob")
                nc.vector.tensor_tensor(
                    ob[:], gel[:], xp[:, cc, 1 + hb * 16 : 1 + hb * 16 + 16, 1:33],
                    OP.add,
                )
                nc.scalar.dma_start(out=out_d[cc][:, hs], in_=ob[:])

    nc.compile()
    return nc


def _host_prep(inputs):
    x = np.asarray(inputs["x"], np.float32)
    proj_w = np.asarray(inputs["proj_w"], np.float32)
    proj_b = np.asarray(inputs["proj_b"], np.float32)
    offset_w = np.asarray(inputs["offset_w"], np.float32)
    offset_b = np.asarray(inputs["offset_b"], np.float32)
    mask_w = np.asarray(inputs["mask_w"], np.float32)
    mask_b = np.asarray(inputs["mask_b"], np.float32)
    gamma = np.asarray(inputs["gamma"], np.float32)
    beta = np.asarray(inputs["beta"], np.float32)

    bf = ml_dtypes.bfloat16
    x16 = x.reshape(B, CC, 128, HW).astype(bf)

    # proj weights -> [k, cc, c128, o]
    # [kg, c128, k-in-group, cc, o] with contraction channel c on dim1:
    # c = cc*128 + c128
    wproj = np.ascontiguousarray(
        proj_w.reshape(C, CC, 128, 3, 3)      # [o, cc, c128, ky, kx]
        .transpose(3, 4, 2, 1, 0)             # [ky, kx, c128, cc, o]
        .reshape(3, 3, 128, CC, C)            # [kg, kig, c128, cc, o]
        .transpose(0, 2, 1, 3, 4)             # [kg, c128, kig, cc, o]
        .reshape(3, 128, 3 * CC * C)
    ).astype(bf)
    # dy taps rows 0-8, dx rows 9-17, mask rows 18-26
    ow = offset_w.reshape(K, 2, C, K)
    om_w = np.concatenate([ow[:, 0], ow[:, 1], mask_w.reshape(K, C, K)], axis=0)
    wom = om_w.transpose(2, 1, 0).reshape(K, CC, 128, 27).astype(bf)
    ob = offset_b.reshape(K, 2)
    bom = np.concatenate([ob[:, 0], ob[:, 1], mask_b]).reshape(27, 1).astype(np.float32)

    hh, ww = np.meshgrid(np.arange(H), np.arange(W), indexing="ij")
    gb = np.zeros((18, HW), np.float32)
    for k in range(K):
        ki, kj = k // 3, k % 3
        gb[k] = (hh + ki - 1 + PAD).reshape(-1)
        gb[9 + k] = (ww + kj - 1 + PAD).reshape(-1)

    pb = proj_b.reshape(CC, 128).astype(np.float32)
    gam2 = gamma.reshape(CC, 128).astype(np.float32)
    bet2 = beta.reshape(CC, 128).astype(np.float32)
    ident = np.eye(128, dtype=bf)

    shared = {
        "wproj": wproj,
        "wom": wom,
        "bom": bom,
        "gb16": gb,
        "pbias": pb,
        "gamma2": gam2,
        "beta2": bet2,
        "ident": ident,
    }
    in_maps = []
    for b in range(B):
        m = dict(shared)
        m["x16"] = x16[b]
        in_maps.append(m)
    return in_maps


def kernel(**inputs):
    if "nc" not in _CACHE:
        _CACHE["nc"] = _build_program()
    nc = _CACHE["nc"]
    in_maps = _host_prep(inputs)
    res = run_bass_kernel_spmd(nc, in_maps, list(range(N_CORES)))
    out = np.stack(
        [r["out"].astype(np.float32).reshape(C, H, W) for r in res.results]
    )
    return out


if __name__ == "__main__":
    nc = _build_program()
    print("program built OK;", len(nc.m.functions[0].blocks), "blocks")


# revision 14
# speedup vs baseline: 5.5603x; 1.0052x over previous
"""Trainium2 Bass kernel for DeformableConv2 block (offset/mask conv ->
modulated deformable conv -> SyncBN -> GELU -> residual).

Sharding: data-parallel over batch B=8 across 8 cores (1 image/core),
weights replicated, BN statistics all-reduced (SyncBatchNorm).

Pipeline per core (image b):
  1. x loaded once as [128, 6, 34, 34] zero-padded bf16 (xp). PE
     transposes build a pixel-major padded DRAM copy xd[2688, 768]
     (rows = padded pixels, 768 channels contiguous per row).
  2. offset/mask 3x3 conv as 54 accumulated bf16 matmuls from strided
     views of xp, fp32 PSUM [27, 1024].
  3. Small-tensor math produces: int16 gather indices (top-left padded
     pixel of each bilinear 2x2 patch) in the 16-partition-wrapped
     layout, and 4 mask-folded bilinear weights (a00,a01,a10,a11) in
     DRAM for per-block broadcast.
  4. HWDGE dma_gather(transpose=True, elem=1536, step=768): one call
     per (half-image, tap, top/bottom) pulls 512 horizontal pixel
     pairs across all 768 channels, transposed to channel-partition
     layout [128, 12, 512]. This runs on the DMA engines at HBM
     bandwidth instead of the Q7 cores (the old ap_gather bottleneck).
  5. DVE combine (5 big ops per (half, tap)) folds the 4 bilinear
     weights + mask: R[128, 6, 512] bf16.
  6. PE contracts wproj[o,c,k] against R: 36 matmuls per (half, tap),
     fp32 PSUM, 6 banks (one per 128-wide output-channel chunk).
  7. Per-channel sum/sumsq via ACT accum_out; [128,12] AllReduce
     across the 8 cores; normalize + erf-GELU + residual (from xp).
"""

import sys

sys.path.insert(0, "/opt/trn_rl_repo")

from contextlib import ExitStack

import ml_dtypes
import numpy as np

import bass_rust
import concourse.bacc as bacc
import concourse.bass as bass
import concourse.tile as tile
from concourse import mybir
from concourse.bass_utils import run_bass_kernel_spmd

F32 = mybir.dt.float32
BF16 = mybir.dt.bfloat16
I16 = mybir.dt.int16
I32 = mybir.dt.int32
AF = mybir.ActivationFunctionType
OP = mybir.AluOpType

B, C, H, W = 8, 768, 32, 32
CC = C // 128            # 6 channel chunks
HW = H * W               # 1024
K = 9                    # 3x3 taps
PAD = 9                  # sample coords in [-9, 40] -> padded rows [0, 50]
PADC = 52                # padded row stride (51 cols + 1 for x+1 pair)
XD_ROWS = 2688           # 21*128 (>= 51*52 + slack for +52 B-view)
VIEW_ROWS = 2600         # gather view row count (> max idx 2597)
BLK = 512                # hw block (matmul moving dim / gather call size)
NB = HW // BLK           # 2
KGS = [(0, 3), (3, 6), (6, 9)]
EPS = 1e-5
N_CORES = 8

_CACHE = {}


def _build_program(mock_cc=False):
    nc = bacc.Bacc("TRN2", target_bir_lowering=False)

    # ---- DRAM I/O ----
    x_d = nc.dram_tensor("x16", [CC, 128, HW], BF16, kind="ExternalInput")
    wproj_d = nc.dram_tensor("wproj", [3, 128, 3 * CC * C], BF16, kind="ExternalInput")
    wom_d = nc.dram_tensor("wom", [K, CC, 128, 96], BF16, kind="ExternalInput")
    bom_d = nc.dram_tensor("bom", [96, 1], F32, kind="ExternalInput")
    gb_d = nc.dram_tensor("gb16", [41, HW], F32, kind="ExternalInput")
    pb_d = nc.dram_tensor("pbias", [CC, 128], F32, kind="ExternalInput")
    gam_d = nc.dram_tensor("gamma2", [CC, 128], F32, kind="ExternalInput")
    bet_d = nc.dram_tensor("beta2", [CC, 128], F32, kind="ExternalInput")
    id_d = nc.dram_tensor("ident", [128, 128], BF16, kind="ExternalInput")
    out_d = nc.dram_tensor("out", [CC, 128, HW], BF16, kind="ExternalOutput")

    with tile.TileContext(nc) as tc, ExitStack() as ctx:
        cst = ctx.enter_context(tc.tile_pool(name="cst", bufs=1))
        dram = ctx.enter_context(tc.tile_pool(name="dram", bufs=1, space="DRAM"))
        actx = ExitStack()  # phase A/B scratch (closed before main loop)
        sm = actx.enter_context(tc.tile_pool(name="sm", bufs=9))
        pconv = actx.enter_context(tc.tile_pool(name="pconv", bufs=1, space="PSUM"))
        pt = actx.enter_context(tc.tile_pool(name="pt", bufs=2, space="PSUM"))
        xtp = actx.enter_context(tc.tile_pool(name="xtp", bufs=2))

        # ---- constants / image ----
        xsb = xtp.tile([128, CC, HW], BF16, tag="xsb")
        nc.sync.dma_start(out=xsb[:], in_=x_d.ap().transpose([1, 0, 2]))
        xp = cst.tile([128, CC, 34, 34], BF16)
        nc.vector.memset(xp[:], 0.0)
        nc.vector.tensor_copy(
            xp[:, :, 1:33, 1:33],
            xsb[:].rearrange("p c (y x) -> p c y x", y=32),
        )
        womsb = cst.tile([128, K, CC, 96], BF16)
        nc.sync.dma_start(out=womsb[:], in_=wom_d.ap().transpose([2, 0, 1, 3]))
        bom = cst.tile([96, 1], F32)
        nc.sync.dma_start(out=bom[:], in_=bom_d[:])
        gb = cst.tile([41, HW], F32)
        nc.sync.dma_start(out=gb[:], in_=gb_d[:])
        pb = cst.tile([128, CC], F32)
        nc.sync.dma_start(out=pb[:], in_=pb_d.ap().transpose([1, 0]))
        gam = cst.tile([128, CC], F32)
        nc.sync.dma_start(out=gam[:], in_=gam_d.ap().transpose([1, 0]))
        bet = cst.tile([128, CC], F32)
        nc.sync.dma_start(out=bet[:], in_=bet_d.ap().transpose([1, 0]))
        ident = cst.tile([128, 128], BF16)
        nc.sync.dma_start(out=ident[:], in_=id_d[:])

        # ---- offset/mask conv: psum27[oc, hw] over 54 (cc,k) matmuls ----
        psum96 = pconv.tile([96, HW], F32)
        for cc in range(CC):
            for k in range(K):
                ki, kj = k // 3, k % 3
                for h in range(2):
                    nc.tensor.matmul(
                        psum96[:, h * BLK : (h + 1) * BLK],
                        lhsT=womsb[:, k, cc, :],
                        rhs=xp[:, cc, ki + h * 16 : ki + h * 16 + 16, kj : kj + 32],
                        start=(cc == 0 and k == 0),
                        stop=(cc == CC - 1 and k == K - 1),
                    )

        # ---- pixel-major padded DRAM image xd[2688, 768] ----
        xd = dram.tile([XD_ROWS, C], BF16)
        zt = cst.tile([128, 4 * C], BF16)
        nc.vector.memset(zt[:], 0.0)
        for j in range(XD_ROWS // 512):
            eng = nc.sync if j % 2 == 0 else nc.scalar
            eng.dma_start(
                out=xd[j * 512 : (j + 1) * 512, :].rearrange(
                    "(p r) c -> p r c", p=128, r=4
                ),
                in_=zt[:].rearrange("p (r c) -> p r c", r=4),
            )
        nc.scalar.dma_start(
            out=xd[2560:2688, :], in_=zt[:, 0:C]
        )
        for pb8 in range(8):
            xt = xtp.tile([128, C], BF16, tag="xt", name=f"xt{pb8}")
            for cc in range(CC):
                pst = pt.tile([128, 128], BF16, tag="pst", name=f"pst{pb8}_{cc}")
                nc.tensor.transpose(
                    pst[:],
                    xsb[:, cc, pb8 * 128 : (pb8 + 1) * 128],
                    ident[:],
                )
                nc.scalar.activation(
                    xt[:, cc * 128 : (cc + 1) * 128], pst[:], AF.Identity
                )
            for a in range(4):
                r0 = (PAD + pb8 * 4 + a) * PADC + PAD
                nc.sync.dma_start(
                    out=xd[r0 : r0 + 32, :], in_=xt[a * 32 : (a + 1) * 32, :]
                )

        # ---- small-tensor math ----
        # psum rows (32-aligned groups): dy taps 0-8, dx 32-40, mask 64-72
        omx = sm.tile([96, HW], F32, tag="omx", bufs=1)
        nc.scalar.activation(omx[:], psum96[:], AF.Identity, bias=bom[:])
        doff = sm.tile([41, HW], F32, tag="s4")
        nc.vector.tensor_scalar(doff[:], omx[0:41, :], 8.0, -8.0, OP.min, OP.max)
        s16 = sm.tile([41, HW], F32, tag="s4")
        nc.vector.tensor_tensor(s16[:], doff[:], gb[:], OP.add)
        i32 = sm.tile([41, HW], I32, tag="s4")
        nc.vector.tensor_copy(i32[:], s16[:])
        fint = sm.tile([41, HW], F32, tag="s4")
        nc.vector.tensor_copy(fint[:], i32[:])
        corr = sm.tile([41, HW], F32, tag="s4")
        nc.vector.tensor_tensor(corr[:], fint[:], s16[:], OP.is_gt)
        ffc = sm.tile([41, HW], F32, tag="s4")
        nc.vector.tensor_tensor(ffc[:], fint[:], corr[:], OP.subtract)
        frac = sm.tile([41, HW], F32, tag="s4")
        nc.vector.tensor_tensor(frac[:], s16[:], ffc[:], OP.subtract)
        u1 = sm.tile([41, HW], F32, tag="s4")
        nc.vector.tensor_scalar(u1[:], frac[:], -1.0, 1.0, OP.mult, OP.add)
        m2 = sm.tile([9, HW], F32, tag="s4")
        nc.scalar.activation(m2[:], omx[64:73, :], AF.Sigmoid)
        # re-base the x-role rows (base 32) to base 0 for two-input ops
        xrb = sm.tile([9, 3, HW], F32, tag="xrb", bufs=1)
        nc.vector.tensor_copy(xrb[:, 0, :], u1[32:41, :])
        nc.vector.tensor_copy(xrb[:, 1, :], frac[32:41, :])
        nc.vector.tensor_copy(xrb[:, 2, :], ffc[32:41, :])
        # gather index first (gates the gathers): idx = ffc_y*52 + ffc_x
        idxf = sm.tile([9, HW], F32, tag="s4")
        nc.vector.scalar_tensor_tensor(
            idxf[:], ffc[0:9, :], float(PADC), xrb[:, 2, :], OP.mult, OP.add
        )
        idx16 = sm.tile([9, HW], I16, tag="s4")
        nc.vector.tensor_copy(idx16[:], idxf[:])

        # wrapped layout: idxw[p, s] = flat[16*s + p%16], replicated x8;
        # column blocks of 32 per (b2, k).
        idxw = cst.tile([128, NB * K * 32], I16)
        for b2 in range(NB):
            for k in range(K):
                eng1 = nc.sync if k % 2 == 0 else nc.scalar
                eng2 = nc.scalar if k % 2 == 0 else nc.sync
                t1w = sm.tile([32, 16], I16, tag="t1w", name="t1w", bufs=4)
                eng1.dma_start(
                    out=t1w[:],
                    in_=idx16[k : k + 1, b2 * BLK : (b2 + 1) * BLK].rearrange(
                        "o (h r) -> o h r", h=32, r=16
                    ),
                )
                t2w = sm.tile([32, 128], I16, tag="t2w", name="t2w", bufs=4)
                nc.vector.tensor_copy(
                    t2w[:].rearrange("h (g r) -> h g r", g=8, r=16),
                    t1w[:].unsqueeze(1).broadcast_to((32, 8, 16)),
                )
                cb = (b2 * K + k) * 32
                nc.sync.dma_start(
                    out=idxw[:, cb : cb + 32], in_=t2w[:], transpose=True
                )

        wA = sm.tile([9, HW], F32, tag="s4")
        nc.vector.scalar_tensor_tensor(wA[:], m2[:], 2.0, u1[0:9, :], OP.mult, OP.mult)
        wB = sm.tile([9, HW], F32, tag="s4")
        nc.vector.scalar_tensor_tensor(wB[:], m2[:], 2.0, frac[0:9, :], OP.mult, OP.mult)
        a4 = sm.tile([9, 4, HW], BF16, tag="a4", bufs=1)
        nc.vector.tensor_tensor(a4[:, 0, :], wA[:], xrb[:, 0, :], OP.mult)
        nc.vector.tensor_tensor(a4[:, 1, :], wA[:], xrb[:, 1, :], OP.mult)
        nc.vector.tensor_tensor(a4[:, 2, :], wB[:], xrb[:, 0, :], OP.mult)
        nc.vector.tensor_tensor(a4[:, 3, :], wB[:], xrb[:, 1, :], OP.mult)
        mwd = dram.tile([K, 4, HW], BF16)
        nc.scalar.dma_start(out=mwd[:], in_=a4[:])

        actx.close()

        # gather source views: rows of 1536 elems (2 pixels) at stride 768
        srcT = xd[:].copy()
        srcT.ap = bass_rust.VecI64Pair([(C, VIEW_ROWS), (1, 2 * C)])
        srcB = xd[PADC:, :].copy()
        srcB.ap = bass_rust.VecI64Pair([(C, VIEW_ROWS), (1, 2 * C)])

        mctx = ExitStack()
        vpool = mctx.enter_context(tc.tile_pool(name="vp", bufs=2))
        rpool = mctx.enter_context(tc.tile_pool(name="rp", bufs=2))
        mpool = mctx.enter_context(tc.tile_pool(name="mp", bufs=2))
        wpool = mctx.enter_context(tc.tile_pool(name="wp", bufs=2))
        pmain = mctx.enter_context(tc.tile_pool(name="pmain", bufs=1, space="PSUM"))

        ysb = cst.tile([128, CC, HW], BF16)
        stats = cst.tile([128, 4 * CC], F32)  # [S_b0|S_b1|Q_b0|Q_b1]
        sqscr = cst.tile([128, BLK], F32)

        # ---- main loop: gather / combine / matmul ----
        for b2 in range(NB):
            psums = [
                pmain.tile([128, BLK], F32, tag=f"ps{o}", name=f"psum_b{b2}_o{o}")
                for o in range(CC)
            ]
            for kg0, kg1 in KGS:
                wt = wpool.tile([128, 3, CC, C], BF16, tag="wt", name="wt")
                nc.sync.dma_start(
                    out=wt[:].rearrange("p a c o -> p (a c o)"),
                    in_=wproj_d[kg0 // 3],
                )
                mbT = mpool.tile([128, 2, 3, BLK], BF16, tag="mT", name="mbT")
                mbB = mpool.tile([128, 2, 3, BLK], BF16, tag="mB", name="mbB")
                for j in range(2):
                    nc.scalar.dma_start(
                        out=mbT[:, j],
                        in_=mwd[kg0:kg1, j, b2 * BLK : (b2 + 1) * BLK]
                        .unsqueeze(0)
                        .broadcast_to((128, 3, BLK)),
                    )
                    nc.scalar.dma_start(
                        out=mbB[:, j],
                        in_=mwd[kg0:kg1, 2 + j, b2 * BLK : (b2 + 1) * BLK]
                        .unsqueeze(0)
                        .broadcast_to((128, 3, BLK)),
                    )
                for k in range(kg0, kg1):
                    cb = (b2 * K + k) * 32
                    vT = vpool.tile([128, 2, CC, BLK], BF16, tag="vT", name="vT")
                    nc.gpsimd.dma_gather(
                        vT[:].rearrange("p a c n -> p (a c) n"),
                        srcT, idxw[:, cb : cb + 32],
                        BLK, BLK, 2 * C, elem_step=C, transpose=True,
                    )
                    vB = vpool.tile([128, 2, CC, BLK], BF16, tag="vB", name="vB")
                    nc.gpsimd.dma_gather(
                        vB[:].rearrange("p a c n -> p (a c) n"),
                        srcB, idxw[:, cb : cb + 32],
                        BLK, BLK, 2 * C, elem_step=C, transpose=True,
                    )
                    mTk = (
                        mbT[:, :, k - kg0, :]
                        .unsqueeze(2)
                        .broadcast_to((128, 2, CC, BLK))
                    )
                    mBk = (
                        mbB[:, :, k - kg0, :]
                        .unsqueeze(2)
                        .broadcast_to((128, 2, CC, BLK))
                    )
                    nc.vector.tensor_tensor(vT[:], vT[:], mTk, OP.mult)
                    nc.vector.tensor_tensor(vB[:], vB[:], mBk, OP.mult)
                    R = rpool.tile([128, CC, BLK], BF16, tag="R", name="R")
                    nc.vector.tensor_tensor(R[:], vT[:, 0], vT[:, 1], OP.add)
                    tmpB = rpool.tile([128, CC, BLK], BF16, tag="tB", name="tB")
                    nc.vector.tensor_tensor(tmpB[:], vB[:, 0], vB[:, 1], OP.add)
                    nc.vector.tensor_tensor(R[:], R[:], tmpB[:], OP.add)
                    for cc in range(CC):
                        for o in range(CC):
                            nc.tensor.matmul(
                                psums[o][:],
                                lhsT=wt[:, k - kg0, cc, o * 128 : (o + 1) * 128],
                                rhs=R[:, cc, :],
                                start=(k == 0 and cc == 0),
                                stop=(k == K - 1 and cc == CC - 1),
                            )
            for o in range(CC):
                nc.scalar.activation(
                    ysb[:, o, b2 * BLK : (b2 + 1) * BLK],
                    psums[o][:],
                    AF.Identity,
                    bias=pb[:, o : o + 1],
                    accum_out=stats[:, b2 * CC + o : b2 * CC + o + 1],
                )
                nc.scalar.activation(
                    sqscr[:],
                    ysb[:, o, b2 * BLK : (b2 + 1) * BLK],
                    AF.Square,
                    accum_out=stats[:, (2 + b2) * CC + o : (2 + b2) * CC + o + 1],
                )

        mctx.close()
        opool = ctx.enter_context(tc.tile_pool(name="op", bufs=2))
        smf = ctx.enter_context(tc.tile_pool(name="smf", bufs=4))

        # ---- SyncBN stats all-reduce ----
        ssum = smf.tile([128, 2 * CC], F32, tag="f")
        nc.vector.tensor_tensor(
            ssum[:, 0:CC], stats[:, 0:CC], stats[:, CC : 2 * CC], OP.add
        )
        nc.vector.tensor_tensor(
            ssum[:, CC : 2 * CC],
            stats[:, 2 * CC : 3 * CC],
            stats[:, 3 * CC : 4 * CC],
            OP.add,
        )
        statloc = dram.tile([128, 2 * CC], F32)
        statglob = dram.tile([128, 2 * CC], F32, addr_space="Shared")
        nc.sync.dma_start(out=statloc[:], in_=ssum[:])
        if mock_cc:
            nc.sync.dma_start(out=statglob[:], in_=statloc[:])
        else:
            nc.gpsimd.collective_compute(
                "AllReduce",
                OP.add,
                replica_groups=[list(range(N_CORES))],
                ins=[statloc[:]],
                outs=[statglob[:]],
            )
        gst = smf.tile([128, 2 * CC], F32, tag="f")
        nc.sync.dma_start(out=gst[:], in_=statglob[:])

        inv_n = 1.0 / (B * HW)
        mean = smf.tile([128, CC], F32, tag="f")
        nc.vector.tensor_scalar(mean[:], gst[:, 0:CC], inv_n, None, OP.mult)
        ex2 = smf.tile([128, CC], F32, tag="f")
        nc.vector.tensor_scalar(ex2[:], gst[:, CC : 2 * CC], inv_n, None, OP.mult)
        var = smf.tile([128, CC], F32, tag="f")
        nc.vector.scalar_tensor_tensor(var[:], mean[:], 1.0, mean[:], OP.mult, OP.mult)
        nc.vector.tensor_tensor(var[:], ex2[:], var[:], OP.subtract)
        epst = smf.tile([128, 1], F32, tag="f")
        nc.vector.memset(epst[:], EPS)
        std = smf.tile([128, CC], F32, tag="f")
        nc.scalar.activation(std[:], var[:], AF.Sqrt, bias=epst[:])
        inv = smf.tile([128, CC], F32, tag="f")
        nc.vector.reciprocal(inv[:], std[:])
        scl = smf.tile([128, CC], F32, tag="f")
        nc.vector.tensor_tensor(scl[:], gam[:], inv[:], OP.mult)
        sft = smf.tile([128, CC], F32, tag="f")
        nc.vector.tensor_tensor(sft[:], mean[:], scl[:], OP.mult)
        nc.vector.tensor_tensor(sft[:], bet[:], sft[:], OP.subtract)

        # ---- normalize + erf-GELU + residual ----
        for cc in range(CC):
            yn = opool.tile([128, HW], F32, tag="yn", name="yn")
            nc.vector.tensor_scalar(
                yn[:],
                ysb[:, cc, :],
                scl[:, cc : cc + 1],
                sft[:, cc : cc + 1],
                OP.mult,
                OP.add,
            )
            gel = opool.tile([128, HW], F32, tag="gel", name="gel")
            nc.scalar.activation(gel[:], yn[:], AF.Gelu)
            ob = opool.tile([128, HW], BF16, tag="ob", name="ob")
            nc.vector.tensor_tensor(
                ob[:].rearrange("p (y x) -> p y x", y=32),
                gel[:].rearrange("p (y x) -> p y x", y=32),
                xp[:, cc, 1:33, 1:33],
                OP.add,
            )
            nc.scalar.dma_start(out=out_d[cc], in_=ob[:])

    nc.compile()
    return nc


def _host_prep(inputs):
    x = np.asarray(inputs["x"], np.float32)
    proj_w = np.asarray(inputs["proj_w"], np.float32)
    proj_b = np.asarray(inputs["proj_b"], np.float32)
    offset_w = np.asarray(inputs["offset_w"], np.float32)
    offset_b = np.asarray(inputs["offset_b"], np.float32)
    mask_w = np.asarray(inputs["mask_w"], np.float32)
    mask_b = np.asarray(inputs["mask_b"], np.float32)
    gamma = np.asarray(inputs["gamma"], np.float32)
    beta = np.asarray(inputs["beta"], np.float32)

    bf = ml_dtypes.bfloat16
    x16 = x.reshape(B, CC, 128, HW).astype(bf)

    # proj weights -> [k, cc, c128, o]
    # [kg, c128, k-in-group, cc, o] with contraction channel c on dim1:
    # c = cc*128 + c128
    wproj = np.ascontiguousarray(
        proj_w.reshape(C, CC, 128, 3, 3)      # [o, cc, c128, ky, kx]
        .transpose(3, 4, 2, 1, 0)             # [ky, kx, c128, cc, o]
        .reshape(3, 3, 128, CC, C)            # [kg, kig, c128, cc, o]
        .transpose(0, 2, 1, 3, 4)             # [kg, c128, kig, cc, o]
        .reshape(3, 128, 3 * CC * C)
    ).astype(bf)
    # dy taps rows 0-8, dx rows 9-17, mask rows 18-26
    ow = offset_w.reshape(K, 2, C, K)
    om_w = np.zeros((96, C, K), np.float32)
    om_w[0:9] = ow[:, 0]
    om_w[32:41] = ow[:, 1]
    om_w[64:73] = mask_w.reshape(K, C, K)
    wom = om_w.transpose(2, 1, 0).reshape(K, CC, 128, 96).astype(bf)
    ob = offset_b.reshape(K, 2)
    bom = np.zeros((96, 1), np.float32)
    bom[0:9, 0] = ob[:, 0]
    bom[32:41, 0] = ob[:, 1]
    bom[64:73, 0] = mask_b

    hh, ww = np.meshgrid(np.arange(H), np.arange(W), indexing="ij")
    gb = np.zeros((41, HW), np.float32)
    for k in range(K):
        ki, kj = k // 3, k % 3
        gb[k] = (hh + ki - 1 + PAD).reshape(-1)
        gb[32 + k] = (ww + kj - 1 + PAD).reshape(-1)

    pb = proj_b.reshape(CC, 128).astype(np.float32)
    gam2 = gamma.reshape(CC, 128).astype(np.float32)
    bet2 = beta.reshape(CC, 128).astype(np.float32)
    ident = np.eye(128, dtype=bf)

    shared = {
        "wproj": wproj,
        "wom": wom,
        "bom": bom,
        "gb16": gb,
        "pbias": pb,
        "gamma2": gam2,
        "beta2": bet2,
        "ident": ident,
    }
    in_maps = []
    for b in range(B):
        m = dict(shared)
        m["x16"] = x16[b]
        in_maps.append(m)
    return in_maps


def kernel(**inputs):
    if "nc" not in _CACHE:
        _CACHE["nc"] = _build_program()
    nc = _CACHE["nc"]
    in_maps = _host_prep(inputs)
    res = run_bass_kernel_spmd(nc, in_maps, list(range(N_CORES)))
    out = np.stack(
        [r["out"].astype(np.float32).reshape(C, H, W) for r in res.results]
    )
    return out


if __name__ == "__main__":
    nc = _build_program()
    print("program built OK;", len(nc.m.functions[0].blocks), "blocks")


# revision 15
# speedup vs baseline: 6.0511x; 1.0883x over previous
"""Trainium2 Bass kernel for DeformableConv2 block (offset/mask conv ->
modulated deformable conv -> SyncBN -> GELU -> residual).

Sharding: data-parallel over batch B=8 across 8 cores (1 image/core),
weights replicated, BN statistics all-reduced (SyncBatchNorm).

Pipeline per core (image b):
  1. x loaded once as [128, 6, 34, 34] zero-padded bf16 (xp). PE
     transposes build a pixel-major padded DRAM copy xd[2688, 768]
     (rows = padded pixels, 768 channels contiguous per row).
  2. offset/mask 3x3 conv as 54 accumulated bf16 matmuls from strided
     views of xp, fp32 PSUM [27, 1024].
  3. Small-tensor math produces: int16 gather indices (top-left padded
     pixel of each bilinear 2x2 patch) in the 16-partition-wrapped
     layout, and 4 mask-folded bilinear weights (a00,a01,a10,a11) in
     DRAM for per-block broadcast.
  4. HWDGE dma_gather(transpose=True, elem=1536, step=768): one call
     per (half-image, tap, top/bottom) pulls 512 horizontal pixel
     pairs across all 768 channels, transposed to channel-partition
     layout [128, 12, 512]. This runs on the DMA engines at HBM
     bandwidth instead of the Q7 cores (the old ap_gather bottleneck).
  5. DVE combine (5 big ops per (half, tap)) folds the 4 bilinear
     weights + mask: R[128, 6, 512] bf16.
  6. PE contracts wproj[o,c,k] against R: 36 matmuls per (half, tap),
     fp32 PSUM, 6 banks (one per 128-wide output-channel chunk).
  7. Per-channel sum/sumsq via ACT accum_out; [128,12] AllReduce
     across the 8 cores; normalize + erf-GELU + residual (from xp).
"""

import sys

sys.path.insert(0, "/opt/trn_rl_repo")

from contextlib import ExitStack

import ml_dtypes
import numpy as np

import bass_rust
import concourse.bacc as bacc
import concourse.bass as bass
import concourse.tile as tile
from concourse import mybir
from concourse.bass_utils import run_bass_kernel_spmd

F32 = mybir.dt.float32
BF16 = mybir.dt.bfloat16
I16 = mybir.dt.int16
I32 = mybir.dt.int32
AF = mybir.ActivationFunctionType
OP = mybir.AluOpType

B, C, H, W = 8, 768, 32, 32
CC = C // 128            # 6 channel chunks
HW = H * W               # 1024
K = 9                    # 3x3 taps
PAD = 9                  # sample coords in [-9, 40] -> padded rows [0, 50]
PADC = 52                # padded row stride (51 cols + 1 for x+1 pair)
XD_ROWS = 2688           # 21*128 (>= 51*52 + slack for +52 B-view)
VIEW_ROWS = 2600         # gather view row count (> max idx 2597)
BLK = 512                # hw block (matmul moving dim / gather call size)
NB = HW // BLK           # 2
KGS = [(0, 3), (3, 6), (6, 9)]
EPS = 1e-5
N_CORES = 8

_CACHE = {}


def _build_program(mock_cc=False):
    nc = bacc.Bacc("TRN2", target_bir_lowering=False)

    # ---- DRAM I/O ----
    x_d = nc.dram_tensor("x16", [CC, 128, HW], BF16, kind="ExternalInput")
    wproj_d = nc.dram_tensor("wproj", [3, 128, 3 * CC * C], BF16, kind="ExternalInput")
    wom_d = nc.dram_tensor("wom", [K, CC, 128, 96], BF16, kind="ExternalInput")
    bom_d = nc.dram_tensor("bom", [96, 1], F32, kind="ExternalInput")
    gb_d = nc.dram_tensor("gb16", [41, HW], F32, kind="ExternalInput")
    pb_d = nc.dram_tensor("pbias", [CC, 128], F32, kind="ExternalInput")
    gam_d = nc.dram_tensor("gamma2", [CC, 128], F32, kind="ExternalInput")
    bet_d = nc.dram_tensor("beta2", [CC, 128], F32, kind="ExternalInput")
    id_d = nc.dram_tensor("ident", [128, 128], BF16, kind="ExternalInput")
    out_d = nc.dram_tensor("out", [CC, 128, HW], BF16, kind="ExternalOutput")

    with tile.TileContext(nc) as tc, ExitStack() as ctx:
        cst = ctx.enter_context(tc.tile_pool(name="cst", bufs=1))
        dram = ctx.enter_context(tc.tile_pool(name="dram", bufs=1, space="DRAM"))
        actx = ExitStack()  # phase A/B scratch (closed before main loop)
        sm = actx.enter_context(tc.tile_pool(name="sm", bufs=9))
        pconv = actx.enter_context(tc.tile_pool(name="pconv", bufs=1, space="PSUM"))
        pt = actx.enter_context(tc.tile_pool(name="pt", bufs=2, space="PSUM"))
        xtp = actx.enter_context(tc.tile_pool(name="xtp", bufs=2))

        # ---- constants / image ----
        xsb = xtp.tile([128, CC, HW], BF16, tag="xsb")
        nc.sync.dma_start(out=xsb[:], in_=x_d.ap().transpose([1, 0, 2]))
        xp = cst.tile([128, CC, 34, 34], BF16)
        nc.vector.memset(xp[:], 0.0)
        nc.vector.tensor_copy(
            xp[:, :, 1:33, 1:33],
            xsb[:].rearrange("p c (y x) -> p c y x", y=32),
        )
        womsb = cst.tile([128, K, CC, 96], BF16)
        nc.sync.dma_start(out=womsb[:], in_=wom_d.ap().transpose([2, 0, 1, 3]))
        bom = cst.tile([96, 1], F32)
        nc.sync.dma_start(out=bom[:], in_=bom_d[:])
        gb = cst.tile([41, HW], F32)
        nc.sync.dma_start(out=gb[:], in_=gb_d[:])
        pb = cst.tile([128, CC], F32)
        nc.sync.dma_start(out=pb[:], in_=pb_d.ap().transpose([1, 0]))
        gam = cst.tile([128, CC], F32)
        nc.sync.dma_start(out=gam[:], in_=gam_d.ap().transpose([1, 0]))
        bet = cst.tile([128, CC], F32)
        nc.sync.dma_start(out=bet[:], in_=bet_d.ap().transpose([1, 0]))
        ident = cst.tile([128, 128], BF16)
        nc.sync.dma_start(out=ident[:], in_=id_d[:])

        # ---- offset/mask conv: psum27[oc, hw] over 54 (cc,k) matmuls ----
        psum96 = pconv.tile([96, HW], F32)
        for cc in range(CC):
            for k in range(K):
                ki, kj = k // 3, k % 3
                for h in range(2):
                    nc.tensor.matmul(
                        psum96[:, h * BLK : (h + 1) * BLK],
                        lhsT=womsb[:, k, cc, :],
                        rhs=xp[:, cc, ki + h * 16 : ki + h * 16 + 16, kj : kj + 32],
                        start=(cc == 0 and k == 0),
                        stop=(cc == CC - 1 and k == K - 1),
                    )

        # ---- pixel-major padded DRAM image xd[2688, 768] ----
        xd = dram.tile([XD_ROWS, C], BF16)
        zt = cst.tile([128, 4 * C], BF16)
        nc.vector.memset(zt[:], 0.0)
        for j in range(XD_ROWS // 512):
            eng = nc.sync if j % 2 == 0 else nc.scalar
            eng.dma_start(
                out=xd[j * 512 : (j + 1) * 512, :].rearrange(
                    "(p r) c -> p r c", p=128, r=4
                ),
                in_=zt[:].rearrange("p (r c) -> p r c", r=4),
            )
        nc.scalar.dma_start(
            out=xd[2560:2688, :], in_=zt[:, 0:C]
        )
        for pb8 in range(8):
            xt = xtp.tile([128, C], BF16, tag="xt", name=f"xt{pb8}")
            for cc in range(CC):
                pst = pt.tile([128, 128], BF16, tag="pst", name=f"pst{pb8}_{cc}")
                nc.tensor.transpose(
                    pst[:],
                    xsb[:, cc, pb8 * 128 : (pb8 + 1) * 128],
                    ident[:],
                )
                nc.scalar.activation(
                    xt[:, cc * 128 : (cc + 1) * 128], pst[:], AF.Identity
                )
            for a in range(4):
                r0 = (PAD + pb8 * 4 + a) * PADC + PAD
                nc.sync.dma_start(
                    out=xd[r0 : r0 + 32, :], in_=xt[a * 32 : (a + 1) * 32, :]
                )

        # ---- small-tensor math ----
        # psum rows (32-aligned groups): dy taps 0-8, dx 32-40, mask 64-72
        omx = sm.tile([96, HW], F32, tag="omx", bufs=1)
        nc.scalar.activation(omx[:], psum96[:], AF.Identity, bias=bom[:])
        doff = sm.tile([41, HW], F32, tag="s4")
        nc.vector.tensor_scalar(doff[:], omx[0:41, :], 8.0, -8.0, OP.min, OP.max)
        s16 = sm.tile([41, HW], F32, tag="s4")
        nc.vector.tensor_tensor(s16[:], doff[:], gb[:], OP.add)
        i32 = sm.tile([41, HW], I32, tag="s4")
        nc.vector.tensor_copy(i32[:], s16[:])
        fint = sm.tile([41, HW], F32, tag="s4")
        nc.vector.tensor_copy(fint[:], i32[:])
        corr = sm.tile([41, HW], F32, tag="s4")
        nc.vector.tensor_tensor(corr[:], fint[:], s16[:], OP.is_gt)
        ffc = sm.tile([41, HW], F32, tag="s4")
        nc.vector.tensor_tensor(ffc[:], fint[:], corr[:], OP.subtract)
        frac = sm.tile([41, HW], F32, tag="s4")
        nc.vector.tensor_tensor(frac[:], s16[:], ffc[:], OP.subtract)
        u1 = sm.tile([41, HW], F32, tag="s4")
        nc.vector.tensor_scalar(u1[:], frac[:], -1.0, 1.0, OP.mult, OP.add)
        m2 = sm.tile([9, HW], F32, tag="s4")
        nc.scalar.activation(m2[:], omx[64:73, :], AF.Sigmoid)
        # re-base the x-role rows (base 32) to base 0 for two-input ops
        xrb = sm.tile([9, 3, HW], F32, tag="xrb", bufs=1)
        nc.vector.tensor_copy(xrb[:, 0, :], u1[32:41, :])
        nc.vector.tensor_copy(xrb[:, 1, :], frac[32:41, :])
        nc.vector.tensor_copy(xrb[:, 2, :], ffc[32:41, :])
        # gather index first (gates the gathers): idx = ffc_y*52 + ffc_x
        idxf = sm.tile([9, HW], F32, tag="s4")
        nc.vector.scalar_tensor_tensor(
            idxf[:], ffc[0:9, :], float(PADC), xrb[:, 2, :], OP.mult, OP.add
        )
        idx16 = sm.tile([9, HW], I16, tag="s4")
        nc.vector.tensor_copy(idx16[:], idxf[:])

        # wrapped layout: idxw[p, s] = flat[16*s + p%16], replicated x8;
        # column blocks of 32 per (b2, k).
        idxw = cst.tile([128, NB * K * 32], I16)
        for b2 in range(NB):
            for k in range(K):
                eng1 = nc.sync if k % 2 == 0 else nc.scalar
                eng2 = nc.scalar if k % 2 == 0 else nc.sync
                t1w = sm.tile([32, 16], I16, tag="t1w", name="t1w", bufs=4)
                eng1.dma_start(
                    out=t1w[:],
                    in_=idx16[k : k + 1, b2 * BLK : (b2 + 1) * BLK].rearrange(
                        "o (h r) -> o h r", h=32, r=16
                    ),
                )
                t2w = sm.tile([32, 128], I16, tag="t2w", name="t2w", bufs=4)
                nc.vector.tensor_copy(
                    t2w[:].rearrange("h (g r) -> h g r", g=8, r=16),
                    t1w[:].unsqueeze(1).broadcast_to((32, 8, 16)),
                )
                cb = (b2 * K + k) * 32
                for a in range(4):
                    nc.vector.transpose(
                        idxw[32 * a : 32 * a + 32, cb : cb + 32],
                        t2w[:, 32 * a : 32 * a + 32],
                    )

        wA = sm.tile([9, HW], F32, tag="s4")
        nc.vector.scalar_tensor_tensor(wA[:], m2[:], 2.0, u1[0:9, :], OP.mult, OP.mult)
        wB = sm.tile([9, HW], F32, tag="s4")
        nc.vector.scalar_tensor_tensor(wB[:], m2[:], 2.0, frac[0:9, :], OP.mult, OP.mult)
        a4 = sm.tile([9, 4, HW], BF16, tag="a4", bufs=1)
        nc.vector.tensor_tensor(a4[:, 0, :], wA[:], xrb[:, 0, :], OP.mult)
        nc.vector.tensor_tensor(a4[:, 1, :], wA[:], xrb[:, 1, :], OP.mult)
        nc.vector.tensor_tensor(a4[:, 2, :], wB[:], xrb[:, 0, :], OP.mult)
        nc.vector.tensor_tensor(a4[:, 3, :], wB[:], xrb[:, 1, :], OP.mult)
        mwd = dram.tile([K, 4, HW], BF16)
        nc.scalar.dma_start(out=mwd[:], in_=a4[:])

        actx.close()

        # gather source views: rows of 1536 elems (2 pixels) at stride 768
        srcT = xd[:].copy()
        srcT.ap = bass_rust.VecI64Pair([(C, VIEW_ROWS), (1, 2 * C)])
        srcB = xd[PADC:, :].copy()
        srcB.ap = bass_rust.VecI64Pair([(C, VIEW_ROWS), (1, 2 * C)])

        mctx = ExitStack()
        vpool = mctx.enter_context(tc.tile_pool(name="vp", bufs=2))
        rpool = mctx.enter_context(tc.tile_pool(name="rp", bufs=2))
        mpool = mctx.enter_context(tc.tile_pool(name="mp", bufs=2))
        wpool = mctx.enter_context(tc.tile_pool(name="wp", bufs=2))
        pmain = mctx.enter_context(tc.tile_pool(name="pmain", bufs=1, space="PSUM"))

        ysb = cst.tile([128, CC, HW], BF16)
        stats = cst.tile([128, 4 * CC], F32)  # [S_b0|S_b1|Q_b0|Q_b1]
        sqscr = cst.tile([128, BLK], F32)

        # ---- main loop: gather / combine / matmul ----
        for b2 in range(NB):
            psums = [
                pmain.tile([128, BLK], F32, tag=f"ps{o}", name=f"psum_b{b2}_o{o}")
                for o in range(CC)
            ]
            for kg0, kg1 in KGS:
                wt = wpool.tile([128, 3, CC, C], BF16, tag="wt", name="wt")
                nc.sync.dma_start(
                    out=wt[:].rearrange("p a c o -> p (a c o)"),
                    in_=wproj_d[kg0 // 3],
                )
                mbT = mpool.tile([128, 2, 3, BLK], BF16, tag="mT", name="mbT")
                mbB = mpool.tile([128, 2, 3, BLK], BF16, tag="mB", name="mbB")
                for j in range(2):
                    nc.scalar.dma_start(
                        out=mbT[:, j],
                        in_=mwd[kg0:kg1, j, b2 * BLK : (b2 + 1) * BLK]
                        .unsqueeze(0)
                        .broadcast_to((128, 3, BLK)),
                    )
                    nc.scalar.dma_start(
                        out=mbB[:, j],
                        in_=mwd[kg0:kg1, 2 + j, b2 * BLK : (b2 + 1) * BLK]
                        .unsqueeze(0)
                        .broadcast_to((128, 3, BLK)),
                    )
                for k in range(kg0, kg1):
                    cb = (b2 * K + k) * 32
                    vT = vpool.tile([128, 2, CC, BLK], BF16, tag="vT", name="vT")
                    nc.gpsimd.dma_gather(
                        vT[:].rearrange("p a c n -> p (a c) n"),
                        srcT, idxw[:, cb : cb + 32],
                        BLK, BLK, 2 * C, elem_step=C, transpose=True,
                    )
                    vB = vpool.tile([128, 2, CC, BLK], BF16, tag="vB", name="vB")
                    nc.gpsimd.dma_gather(
                        vB[:].rearrange("p a c n -> p (a c) n"),
                        srcB, idxw[:, cb : cb + 32],
                        BLK, BLK, 2 * C, elem_step=C, transpose=True,
                    )
                    mTk = (
                        mbT[:, :, k - kg0, :]
                        .unsqueeze(2)
                        .broadcast_to((128, 2, CC, BLK))
                    )
                    mBk = (
                        mbB[:, :, k - kg0, :]
                        .unsqueeze(2)
                        .broadcast_to((128, 2, CC, BLK))
                    )
                    nc.vector.tensor_tensor(vT[:], vT[:], mTk, OP.mult)
                    nc.vector.tensor_tensor(vB[:], vB[:], mBk, OP.mult)
                    R = rpool.tile([128, CC, BLK], BF16, tag="R", name="R")
                    nc.vector.tensor_tensor(R[:], vT[:, 0], vT[:, 1], OP.add)
                    tmpB = rpool.tile([128, CC, BLK], BF16, tag="tB", name="tB")
                    nc.vector.tensor_tensor(tmpB[:], vB[:, 0], vB[:, 1], OP.add)
                    nc.vector.tensor_tensor(R[:], R[:], tmpB[:], OP.add)
                    for cc in range(CC):
                        for o in range(CC):
                            nc.tensor.matmul(
                                psums[o][:],
                                lhsT=wt[:, k - kg0, cc, o * 128 : (o + 1) * 128],
                                rhs=R[:, cc, :],
                                start=(k == 0 and cc == 0),
                                stop=(k == K - 1 and cc == CC - 1),
                            )
            for o in range(CC):
                nc.scalar.activation(
                    ysb[:, o, b2 * BLK : (b2 + 1) * BLK],
                    psums[o][:],
                    AF.Identity,
                    bias=pb[:, o : o + 1],
                    accum_out=stats[:, b2 * CC + o : b2 * CC + o + 1],
                )
                nc.scalar.activation(
                    sqscr[:],
                    ysb[:, o, b2 * BLK : (b2 + 1) * BLK],
                    AF.Square,
                    accum_out=stats[:, (2 + b2) * CC + o : (2 + b2) * CC + o + 1],
                )

        mctx.close()
        opool = ctx.enter_context(tc.tile_pool(name="op", bufs=2))
        smf = ctx.enter_context(tc.tile_pool(name="smf", bufs=4))

        # ---- SyncBN stats all-reduce ----
        ssum = smf.tile([128, 2 * CC], F32, tag="f")
        nc.vector.tensor_tensor(
            ssum[:, 0:CC], stats[:, 0:CC], stats[:, CC : 2 * CC], OP.add
        )
        nc.vector.tensor_tensor(
            ssum[:, CC : 2 * CC],
            stats[:, 2 * CC : 3 * CC],
            stats[:, 3 * CC : 4 * CC],
            OP.add,
        )
        statloc = dram.tile([128, 2 * CC], F32)
        statglob = dram.tile([128, 2 * CC], F32, addr_space="Shared")
        nc.sync.dma_start(out=statloc[:], in_=ssum[:])
        if mock_cc:
            nc.sync.dma_start(out=statglob[:], in_=statloc[:])
        else:
            nc.gpsimd.collective_compute(
                "AllReduce",
                OP.add,
                replica_groups=[list(range(N_CORES))],
                ins=[statloc[:]],
                outs=[statglob[:]],
            )
        gst = smf.tile([128, 2 * CC], F32, tag="f")
        nc.sync.dma_start(out=gst[:], in_=statglob[:])

        inv_n = 1.0 / (B * HW)
        mean = smf.tile([128, CC], F32, tag="f")
        nc.vector.tensor_scalar(mean[:], gst[:, 0:CC], inv_n, None, OP.mult)
        ex2 = smf.tile([128, CC], F32, tag="f")
        nc.vector.tensor_scalar(ex2[:], gst[:, CC : 2 * CC], inv_n, None, OP.mult)
        var = smf.tile([128, CC], F32, tag="f")
        nc.vector.scalar_tensor_tensor(var[:], mean[:], 1.0, mean[:], OP.mult, OP.mult)
        nc.vector.tensor_tensor(var[:], ex2[:], var[:], OP.subtract)
        epst = smf.tile([128, 1], F32, tag="f")
        nc.vector.memset(epst[:], EPS)
        std = smf.tile([128, CC], F32, tag="f")
        nc.scalar.activation(std[:], var[:], AF.Sqrt, bias=epst[:])
        inv = smf.tile([128, CC], F32, tag="f")
        nc.vector.reciprocal(inv[:], std[:])
        scl = smf.tile([128, CC], F32, tag="f")
        nc.vector.tensor_tensor(scl[:], gam[:], inv[:], OP.mult)
        sft = smf.tile([128, CC], F32, tag="f")
        nc.vector.tensor_tensor(sft[:], mean[:], scl[:], OP.mult)
        nc.vector.tensor_tensor(sft[:], bet[:], sft[:], OP.subtract)

        # ---- normalize + erf-GELU + residual ----
        for cc in range(CC):
            yn = opool.tile([128, HW], F32, tag="yn", name="yn")
            nc.vector.tensor_scalar(
                yn[:],
                ysb[:, cc, :],
                scl[:, cc : cc + 1],
                sft[:, cc : cc + 1],
                OP.mult,
                OP.add,
            )
            gel = opool.tile([128, HW], F32, tag="gel", name="gel")
            nc.scalar.activation(gel[:], yn[:], AF.Gelu)
            ob = opool.tile([128, HW], BF16, tag="ob", name="ob")
            nc.vector.tensor_tensor(
                ob[:].rearrange("p (y x) -> p y x", y=32),
                gel[:].rearrange("p (y x) -> p y x", y=32),
                xp[:, cc, 1:33, 1:33],
                OP.add,
            )
            nc.scalar.dma_start(out=out_d[cc], in_=ob[:])

    nc.compile()
    return nc


def _host_prep(inputs):
    x = np.asarray(inputs["x"], np.float32)
    proj_w = np.asarray(inputs["proj_w"], np.float32)
    proj_b = np.asarray(inputs["proj_b"], np.float32)
    offset_w = np.asarray(inputs["offset_w"], np.float32)
    offset_b = np.asarray(inputs["offset_b"], np.float32)
    mask_w = np.asarray(inputs["mask_w"], np.float32)
    mask_b = np.asarray(inputs["mask_b"], np.float32)
    gamma = np.asarray(inputs["gamma"], np.float32)
    beta = np.asarray(inputs["beta"], np.float32)

    bf = ml_dtypes.bfloat16
    x16 = x.reshape(B, CC, 128, HW).astype(bf)

    # proj weights -> [k, cc, c128, o]
    # [kg, c128, k-in-group, cc, o] with contraction channel c on dim1:
    # c = cc*128 + c128
    wproj = np.ascontiguousarray(
        proj_w.reshape(C, CC, 128, 3, 3)      # [o, cc, c128, ky, kx]
        .transpose(3, 4, 2, 1, 0)             # [ky, kx, c128, cc, o]
        .reshape(3, 3, 128, CC, C)            # [kg, kig, c128, cc, o]
        .transpose(0, 2, 1, 3, 4)             # [kg, c128, kig, cc, o]
        .reshape(3, 128, 3 * CC * C)
    ).astype(bf)
    # dy taps rows 0-8, dx rows 9-17, mask rows 18-26
    ow = offset_w.reshape(K, 2, C, K)
    om_w = np.zeros((96, C, K), np.float32)
    om_w[0:9] = ow[:, 0]
    om_w[32:41] = ow[:, 1]
    om_w[64:73] = mask_w.reshape(K, C, K)
    wom = om_w.transpose(2, 1, 0).reshape(K, CC, 128, 96).astype(bf)
    ob = offset_b.reshape(K, 2)
    bom = np.zeros((96, 1), np.float32)
    bom[0:9, 0] = ob[:, 0]
    bom[32:41, 0] = ob[:, 1]
    bom[64:73, 0] = mask_b

    hh, ww = np.meshgrid(np.arange(H), np.arange(W), indexing="ij")
    gb = np.zeros((41, HW), np.float32)
    for k in range(K):
        ki, kj = k // 3, k % 3
        gb[k] = (hh + ki - 1 + PAD).reshape(-1)
        gb[32 + k] = (ww + kj - 1 + PAD).reshape(-1)

    pb = proj_b.reshape(CC, 128).astype(np.float32)
    gam2 = gamma.reshape(CC, 128).astype(np.float32)
    bet2 = beta.reshape(CC, 128).astype(np.float32)
    ident = np.eye(128, dtype=bf)

    shared = {
        "wproj": wproj,
        "wom": wom,
        "bom": bom,
        "gb16": gb,
        "pbias": pb,
        "gamma2": gam2,
        "beta2": bet2,
        "ident": ident,
    }
    in_maps = []
    for b in range(B):
        m = dict(shared)
        m["x16"] = x16[b]
        in_maps.append(m)
    return in_maps


def kernel(**inputs):
    if "nc" not in _CACHE:
        _CACHE["nc"] = _build_program()
    nc = _CACHE["nc"]
    in_maps = _host_prep(inputs)
    res = run_bass_kernel_spmd(nc, in_maps, list(range(N_CORES)))
    out = np.stack(
        [r["out"].astype(np.float32).reshape(C, H, W) for r in res.results]
    )
    return out


if __name__ == "__main__":
    nc = _build_program()
    print("program built OK;", len(nc.m.functions[0].blocks), "blocks")
